# revision 27
# baseline (speedup 1.0000x reference)
"""Bass/Tile kernel builder for the pre-LN attention block (dense_transformer).

Sharding: 8 cores = 4 batches x 2 query-halves. Each core:
  - loads x for its full batch; per 128-row block: LN1, transpose -> hT
    (dim-major, bf16), V matmuls for that block (PE fills DVE/DMA gaps)
  - attention per head-pair j with the K/Q matmuls for pair j+1 interleaved
    into the kt loop (PE fills the exp/mask bubbles; Act engine is the
    phase-C floor); scores kept transposed [k, q]: no max-subtraction
    (|score| <= ~9), denominator via ones-column appended to V
  - proj + residual (row-major), LN2, MLP, residual, store y rows

Dtypes: bf16 everywhere on the matmul paths (weights incl. proj/MLP), fp32
residuals/stats/denominators. Bias matmuls skipped when biases are all zero
(with_bias=False); host folds gamma/beta into weights either way.
SPMD trick: host rotates rows so each core's own rows are always [0, NQ).

PSUM budget: A: tp1(3x1)+qkvps(2x2)=7 banks; B/C: sps(2x2)+avps(2x2)=8,
K/Q units share the sps pool slots. D: nbps+prps; E: fps+yps.
"""

import sys

sys.path.insert(0, "/opt/trn_rl_repo")

from contextlib import ExitStack

import numpy as np
import ml_dtypes

import concourse.bass as bass
import concourse.tile as tile
import concourse.mybir as mybir
from concourse import bacc

F32 = mybir.dt.float32
F32R = mybir.dt.float32r
BF16 = mybir.dt.bfloat16
AF = mybir.ActivationFunctionType
ALU = mybir.AluOpType

DIM = 768
H = 12
DH = 64
HID = 3072
SCALE = DH ** -0.5
EPS = 1e-6
P = 128
DT = DIM // P


def r(x):
    return x.bitcast(F32R)


def build_nc(S=2048, NQ=1024, mlp_chunk=512, gelu=True, repeat=1, stop_after=None, with_bias=True):
    KT = S // P
    NQT = NQ // P
    assert NQ % 512 == 0
    QC = NQ // 512
    HT = HID // P
    MC = NQ // mlp_chunk
    KCW = min(1024, S)
    QCW = min(1024, NQ)

    nc = bacc.Bacc("TRN2", target_bir_lowering=False, debug=False, num_devices=8)

    dx = nc.dram_tensor("x", [P, S // P, DIM], F32, kind="ExternalInput").ap()
    dmask = nc.dram_tensor("maskT", [P, S // P, NQ], BF16, kind="ExternalInput").ap()
    dwv = nc.dram_tensor("wvr", [P, DT, DIM], BF16, kind="ExternalInput").ap()
    dwkq = nc.dram_tensor("wkqr", [2 * DT, P, DT * P], BF16, kind="ExternalInput").ap()
    dbqkv = nc.dram_tensor("bqkv_pp", [P, 3 * DT], F32, kind="ExternalInput").ap()
    dbv = nc.dram_tensor("bv_row", [1, DIM], F32R, kind="ExternalInput").ap()
    dwproj = nc.dram_tensor("wprojr", [P, DT, DIM], BF16, kind="ExternalInput").ap()
    dbproj = nc.dram_tensor("bproj_row", [1, DIM], F32R, kind="ExternalInput").ap()
    dw1 = nc.dram_tensor("w1r", [HT, P, DT * P], BF16, kind="ExternalInput").ap()
    db1 = nc.dram_tensor("b1_pp", [P, HT], F32, kind="ExternalInput").ap()
    dw2 = nc.dram_tensor("w2", [HID, DIM], BF16, kind="ExternalInput").ap()
    db2 = nc.dram_tensor("b2_row", [1, DIM], F32R, kind="ExternalInput").ap()
    de2map = nc.dram_tensor("e2map", [2, P], F32, kind="ExternalInput").ap()
    dones = nc.dram_tensor("ones_row", [1, P], F32R, kind="ExternalInput").ap()
    dident_bf = nc.dram_tensor("ident_bf", [P, P], BF16, kind="ExternalInput").ap()
    dident_f = nc.dram_tensor("ident_f", [P, P], F32, kind="ExternalInput").ap()
    dy = nc.dram_tensor("y", [P, NQ // P, DIM], F32, kind="ExternalOutput").ap()

    xv = dx
    maskv = dmask
    yv = dy

    with nc.allow_low_precision(
        reason="fp32r matmuls + bf16 attention path validated offline"
    ), tile.TileContext(nc) as tc, ExitStack() as top:
        rep_ctx = tc.For_i(0, repeat, 1) if repeat > 1 else ExitStack()
        top.enter_context(rep_ctx)
        consts = top.enter_context(tc.tile_pool(name="consts", bufs=1))
        ident_bf = consts.tile([P, P], BF16)
        nc.sync.dma_start(out=ident_bf[:], in_=dident_bf[:])
        e2map_sb = consts.tile([2, P], F32)
        nc.gpsimd.dma_start(out=e2map_sb[:], in_=de2map[:])
        eps_t = consts.tile([P, 1], F32)
        nc.vector.memset(eps_t[:], EPS)
        bqkv_pp = consts.tile([P, 3 * DT], F32)
        nc.gpsimd.dma_start(out=bqkv_pp[:], in_=dbqkv[:])
        if with_bias:
            ones_col = consts.tile([1, P], F32)
            nc.gpsimd.dma_start(out=r(ones_col[:]), in_=dones[:])
            bv_row = consts.tile([1, DIM], F32)
            nc.gpsimd.dma_start(out=r(bv_row[:]), in_=dbv[:])
            bproj_row = consts.tile([1, DIM], F32)
            nc.gpsimd.dma_start(out=r(bproj_row[:]), in_=dbproj[:])
            b1_pp = consts.tile([P, HT], F32)
            nc.gpsimd.dma_start(out=b1_pp[:], in_=db1[:])
            b2_row = consts.tile([1, DIM], F32)
            nc.gpsimd.dma_start(out=r(b2_row[:]), in_=db2[:])

        def layer_norm_tile(stats_pool, x_ap, out_ap):
            stats = stats_pool.tile([P, 2, 6], F32, tag="lnstats")
            for sg in range(2):
                nc.vector.bn_stats(
                    out=stats[:, sg, :], in_=x_ap[:, sg * 384 : (sg + 1) * 384]
                )
            mv = stats_pool.tile([P, 2], F32, tag="lnmv")
            nc.vector.bn_aggr(out=mv[:], in_=stats[:])
            sd = stats_pool.tile([P, 1], F32, tag="lnsd")
            nc.scalar.activation(
                out=sd[:], in_=mv[:, 1:2], func=AF.Sqrt, bias=eps_t[:], scale=1.0
            )
            rstd = stats_pool.tile([P, 1], F32, tag="lnrstd")
            nc.vector.reciprocal(out=rstd[:], in_=sd[:])
            nc.vector.tensor_scalar(
                out=out_ap,
                in0=x_ap,
                scalar1=mv[:, 0:1],
                scalar2=rstd[:],
                op0=ALU.subtract,
                op1=ALU.mult,
            )

        wa_pool = top.enter_context(tc.tile_pool(name="wa", bufs=1))
        waT = wa_pool.tile([P, DT, NQ], BF16)
        recip_all = wa_pool.tile([2, H // 2, NQ], F32)

        with ExitStack() as s_kqv:
            kqv_pool = s_kqv.enter_context(tc.tile_pool(name="kqv", bufs=1))
            k_sb = kqv_pool.tile([P, DT, S], BF16)
            q_sb = kqv_pool.tile([P, DT, NQ], BF16)
            v_sb = kqv_pool.tile([P, KT, H * (DH + 1)], BF16)
            mask_sb = kqv_pool.tile([P, KT, NQ], BF16)
            nc.gpsimd.dma_start(out=mask_sb[:], in_=maskv[:])
            v4 = v_sb.rearrange("p t (h s) -> p t h s", s=DH + 1)
            nc.vector.memset(v4[:, :, :, DH : DH + 1], 1.0)

            with ExitStack() as s_ht:
                ht_pool = s_ht.enter_context(tc.tile_pool(name="htp", bufs=1))
                hT = ht_pool.tile([P, DT, S], BF16)

                # ---------- phase A: LN1 + transpose -> hT, V per block ----------
                with ExitStack() as ph:
                    wv_pool = ph.enter_context(tc.tile_pool(name="wv", bufs=1))
                    wv_sb = wv_pool.tile([P, DT, DIM], BF16)
                    nc.gpsimd.dma_start(out=wv_sb[:], in_=dwv[:])
                    xo_pool = ph.enter_context(tc.tile_pool(name="xo", bufs=4))
                    h_pool = ph.enter_context(tc.tile_pool(name="h1", bufs=4))
                    st_pool = ph.enter_context(tc.tile_pool(name="st1", bufs=6))
                    tp_pool = ph.enter_context(
                        tc.tile_pool(name="tp1", bufs=3, space=bass.MemorySpace.PSUM)
                    )
                    qkv_ps = ph.enter_context(
                        tc.tile_pool(name="qkvps", bufs=2, space=bass.MemorySpace.PSUM)
                    )

                    def v_block(t):
                        psv = qkv_ps.tile([P, DIM], F32, tag="qkvps", name=f"psv_{t}")
                        for dt in range(DT):
                            for c0, cw in ((0, 512), (512, 256)):
                                nc.tensor.matmul(
                                    psv[:, c0 : c0 + cw],
                                    lhsT=hT[:, dt, t * P : (t + 1) * P],
                                    rhs=wv_sb[:, dt, c0 : c0 + cw],
                                    start=(dt == 0),
                                    stop=(not with_bias and dt == DT - 1),
                                )
                        if with_bias:
                            for c0, cw in ((0, 512), (512, 256)):
                                nc.tensor.matmul(
                                    psv[:, c0 : c0 + cw],
                                    lhsT=r(ones_col[:, :]),
                                    rhs=r(bv_row[:, c0 : c0 + cw]),
                                    start=False,
                                    stop=True,
                                )
                        nc.vector.tensor_copy(
                            out=v4[:, t, 0:H, 0:DH],
                            in_=psv[:].rearrange("p (h s) -> p h s", s=DH),
                        )

                    for t in range(KT):
                        xo = xo_pool.tile([P, DIM], F32, tag="xo")
                        nc.sync.dma_start(out=xo[:], in_=xv[:, t, :])
                        h_t = h_pool.tile([P, DIM], BF16, tag="h")
                        layer_norm_tile(st_pool, xo[:], h_t[:])
                        tp = tp_pool.tile([P, DIM], BF16, tag="tp", name=f"tp_{t}")
                        for dt in range(DT):
                            nc.tensor.transpose(
                                tp[:, dt * P : (dt + 1) * P],
                                h_t[:, dt * P : (dt + 1) * P],
                                ident_bf[:],
                            )
                        nc.vector.tensor_copy(
                            out=hT[:, :, t * P : (t + 1) * P],
                            in_=tp[:].rearrange("p (d o) -> p d o", o=P),
                        )
                        if t > 0:
                            v_block(t - 1)
                    v_block(KT - 1)

                if stop_after != "ab":
                    # ---------- phase B/C: attention, K/Q(j+1) interleaved ----------
                    # Per (j, hh) pass: scores computed one kt ahead of AV so
                    # the Act engine (exp, the phase floor) always has its
                    # next input ready; K/Q matmuls for pair j+1 are sliced
                    # into 2-matmul pieces dropped between scores and AV.
                    with ExitStack() as ph:
                        wqk_pool = ph.enter_context(tc.tile_pool(name="wqk", bufs=4))
                        s_ps = ph.enter_context(
                            tc.tile_pool(name="sps", bufs=2, space=bass.MemorySpace.PSUM)
                        )
                        av_ps = ph.enter_context(
                            tc.tile_pool(name="avps", bufs=1, space=bass.MemorySpace.PSUM)
                        )
                        kq_ps = ph.enter_context(
                            tc.tile_pool(name="kqps", bufs=1, space=bass.MemorySpace.PSUM)
                        )
                        p_pool = ph.enter_context(tc.tile_pool(name="pp", bufs=5))
                        dtmp_pool = ph.enter_context(tc.tile_pool(name="dtmp", bufs=2))
                        scr_pool = ph.enter_context(tc.tile_pool(name="scr", bufs=1))

                        def load_w(m, qk):
                            w = wqk_pool.tile(
                                [P, DT, P], BF16, tag="wqk", name=f"w{qk}_{m}"
                            )
                            idx = m if qk == "k" else DT + m
                            nc.gpsimd.dma_start(
                                out=w[:],
                                in_=dwkq[idx].rearrange("p (dt o) -> p dt o", o=P),
                            )
                            return w

                        def kq_piece(m, w, qk, cp, dt, unit_box):
                            if dt == 0:
                                unit_box[0] = kq_ps.tile(
                                    [P, 1024], F32, tag="kq", name=f"{qk}u_{m}_{cp}"
                                )
                            ps = unit_box[0]
                            for ci in range(2):
                                c = cp * 2 + ci
                                nc.tensor.matmul(
                                    ps[:, ci * 512 : (ci + 1) * 512],
                                    lhsT=w[:, dt, :],
                                    rhs=hT[:, dt, c * 512 : (c + 1) * 512],
                                    start=(dt == 0),
                                    stop=(dt == DT - 1),
                                )
                            if dt == DT - 1:
                                dst, boff = (k_sb, DT) if qk == "k" else (q_sb, 0)
                                nc.vector.tensor_scalar_add(
                                    out=dst[:, m, cp * 1024 : (cp + 1) * 1024],
                                    in0=ps[:],
                                    scalar1=bqkv_pp[:, boff + m : boff + m + 1],
                                )

                        def kq_full(m, w, qk, cp):
                            box = [None]
                            for dt in range(DT):
                                kq_piece(m, w, qk, cp, dt, box)

                        wk_cur = load_w(0, "k")
                        wq_cur = load_w(0, "q")
                        kq_full(0, wk_cur, "k", 0)
                        kq_full(0, wk_cur, "k", 1)
                        kq_full(0, wq_cur, "q", 0)

                        def psb_piece(jj, c, box):
                            # normalize waT[:, jj] by its softmax denominators:
                            # e2map broadcasts the two recip rows to 128
                            # partitions (plain-fp32 matmul; fp32r would need
                            # an fp32r-rounded producer)
                            if c == 0:
                                box[0] = kq_ps.tile(
                                    [P, NQ], F32, tag="kq", name=f"psb_{jj}"
                                )
                            psb = box[0]
                            nc.tensor.matmul(
                                psb[:, c * 512 : (c + 1) * 512],
                                lhsT=e2map_sb[:, :],
                                rhs=recip_all[:, jj, c * 512 : (c + 1) * 512],
                            )
                            if c == 1:
                                nc.vector.tensor_tensor(
                                    out=waT[:, jj, :],
                                    in0=waT[:, jj, :],
                                    in1=psb[:],
                                    op=ALU.mult,
                                )

                        PIECE_SLOTS = set(range(0, 2 * KT, 2)) | {1, 3}
                        for j in range(H // 2):
                            if j + 1 < H // 2:
                                wk_cur = load_w(j + 1, "k")
                                wq_cur = load_w(j + 1, "q")
                                pieces = [
                                    ("kq", qk, cp, dt)
                                    for qk, cp in (("k", 0), ("k", 1), ("q", 0))
                                    for dt in range(DT)
                                ]
                            else:
                                pieces = [
                                    ("psb", jj, c)
                                    for jj in range(H // 2 - 1)
                                    for c in range(2)
                                ]
                            pc = 0
                            unit_box = [None]
                            den_j = dtmp_pool.tile(
                                [2, NQ], F32, tag="den", name=f"den_{j}"
                            )
                            for hh in range(2):
                                lo, hi = hh * DH, (hh + 1) * DH
                                hgl = (2 * j + hh) * (DH + 1)

                                def scores_mm(kt):
                                    pss = s_ps.tile(
                                        [P, NQ], F32, tag="s",
                                        name=f"s_{j}_{hh}_{kt}",
                                    )
                                    for c in range(QC):
                                        nc.tensor.matmul(
                                            pss[:, c * 512 : (c + 1) * 512],
                                            lhsT=k_sb[lo:hi, j, kt * P : (kt + 1) * P],
                                            rhs=q_sb[lo:hi, j, c * 512 : (c + 1) * 512],
                                            tile_position=(lo, 0),
                                        )
                                    pe_t = p_pool.tile([P, NQ], BF16, tag="pe")
                                    nc.scalar.activation(
                                        out=pe_t[:], in_=pss[:], func=AF.Exp, scale=SCALE
                                    )
                                    pt = p_pool.tile([P, NQ], BF16, tag="p")
                                    nc.vector.tensor_tensor(
                                        out=pt[:],
                                        in0=pe_t[:],
                                        in1=mask_sb[:, kt, :],
                                        op=ALU.mult,
                                    )
                                    return pt

                                av = av_ps.tile(
                                    [P, NQ], F32, tag="av", name=f"av_{j}_{hh}"
                                )
                                pt_cur = scores_mm(0)
                                for kt in range(KT):
                                    pt_next = scores_mm(kt + 1) if kt + 1 < KT else None
                                    if hh * KT + kt in PIECE_SLOTS and pc < len(pieces):
                                        piece = pieces[pc]
                                        if piece[0] == "kq":
                                            _, qk, cp, dt = piece
                                            kq_piece(
                                                j + 1,
                                                wk_cur if qk == "k" else wq_cur,
                                                qk, cp, dt, unit_box,
                                            )
                                        else:
                                            _, jj, c = piece
                                            psb_piece(jj, c, unit_box)
                                        pc += 1
                                    for c in range(QC):
                                        nc.tensor.matmul(
                                            av[0 : DH + 1, c * 512 : (c + 1) * 512],
                                            lhsT=v_sb[:, kt, hgl : hgl + DH + 1],
                                            rhs=pt_cur[:, c * 512 : (c + 1) * 512],
                                            start=(kt == 0),
                                            stop=(kt == KT - 1),
                                        )
                                    pt_cur = pt_next
                                nc.vector.tensor_copy(
                                    out=waT[hh * DH : (hh + 1) * DH, j, :],
                                    in_=av[0:DH, :],
                                )
                                dtmp = dtmp_pool.tile([P, NQ], F32, tag="dtmp")
                                nc.vector.tensor_copy(
                                    out=dtmp[DH : DH + 1, :], in_=av[DH : DH + 1, :]
                                )
                                (nc.sync if hh else nc.gpsimd).dma_start(
                                    out=den_j[hh : hh + 1, :], in_=dtmp[DH : DH + 1, :]
                                )
                            scr_j = scr_pool.tile([2, NQ], F32, tag="scr", name=f"scr_{j}")
                            nc.vector.reciprocal_approx_accurate(
                                out=recip_all[:, j, :], in_=den_j[:], scratch=scr_j[:]
                            )
                        box5 = [None]
                        psb_piece(H // 2 - 1, 0, box5)
                        psb_piece(H // 2 - 1, 1, box5)

        if stop_after is None:
            # ---------- phases D+E ----------
            with ExitStack() as s_de:
                de_pool = s_de.enter_context(tc.tile_pool(name="de", bufs=1))
                x2_sb = de_pool.tile([P, NQT, DIM], F32)
                h2T = de_pool.tile([P, DT, NQ], BF16)

                # ---------- phase D: normalize wa + proj + residual + LN2 ----------
                with ExitStack() as ph:
                    wp_pool = ph.enter_context(tc.tile_pool(name="wp", bufs=1))
                    xr_pool = ph.enter_context(tc.tile_pool(name="xr", bufs=3))
                    wproj_sb = wp_pool.tile([P, DT, DIM], BF16)
                    nc.gpsimd.dma_start(
                        out=wproj_sb[:],
                        in_=dwproj[:],
                    )
                    with ExitStack() as ph2:
                        pr_ps = ph2.enter_context(
                            tc.tile_pool(name="prps", bufs=3, space=bass.MemorySpace.PSUM)
                        )
                        for t in range(NQT):
                            xr = xr_pool.tile([P, DIM], F32, tag="xr")
                            nc.sync.dma_start(out=xr[:], in_=xv[:, t, :])
                            psp = pr_ps.tile([P, DIM], F32, tag="pr")
                            for dt in range(DT):
                                for c0, cw in ((0, 512), (512, 256)):
                                    nc.tensor.matmul(
                                        psp[:, c0 : c0 + cw],
                                        lhsT=waT[:, dt, t * P : (t + 1) * P],
                                        rhs=wproj_sb[:, dt, c0 : c0 + cw],
                                        start=(dt == 0),
                                        stop=(not with_bias and dt == DT - 1),
                                    )
                            if with_bias:
                                for c0, cw in ((0, 512), (512, 256)):
                                    nc.tensor.matmul(
                                        psp[:, c0 : c0 + cw],
                                        lhsT=r(ones_col[:, :]),
                                        rhs=r(bproj_row[:, c0 : c0 + cw]),
                                        start=False,
                                        stop=True,
                                    )
                            nc.vector.tensor_tensor(
                                out=x2_sb[:, t, :], in0=psp[:], in1=xr[:], op=ALU.add
                            )
                    h2_pool = ph.enter_context(tc.tile_pool(name="h2", bufs=4))
                    st2_pool = ph.enter_context(tc.tile_pool(name="st2", bufs=6))
                    tp2_pool = ph.enter_context(
                        tc.tile_pool(name="tp2", bufs=7, space=bass.MemorySpace.PSUM)
                    )
                    for tg in range(NQT // 4):
                        ps = [
                            tp2_pool.tile([P, 512], BF16, tag="tp2", name=f"tp2_{tg}_{i}")
                            for i in range(DT)
                        ]
                        for tt in range(4):
                            t = tg * 4 + tt
                            h2_t = h2_pool.tile([P, DIM], BF16, tag="h2")
                            layer_norm_tile(st2_pool, x2_sb[:, t, :], h2_t[:])
                            for dt in range(DT):
                                nc.tensor.transpose(
                                    ps[dt][:, tt * P : (tt + 1) * P],
                                    h2_t[:, dt * P : (dt + 1) * P],
                                    ident_bf[:],
                                )
                        for dt in range(DT):
                            nc.vector.tensor_copy(
                                out=h2T[:, dt, tg * 512 : (tg + 1) * 512], in_=ps[dt][:]
                            )

                # ---------- phase E: MLP ----------
                with ExitStack() as ph:
                    w1_pool = ph.enter_context(tc.tile_pool(name="w1p", bufs=4))
                    w2_pool = ph.enter_context(tc.tile_pool(name="w2p", bufs=4))
                    g_pool = ph.enter_context(tc.tile_pool(name="gp", bufs=1))
                    f_ps = ph.enter_context(
                        tc.tile_pool(name="fps", bufs=2 if not with_bias else 3, space=bass.MemorySpace.PSUM)
                    )
                    y_ps = ph.enter_context(
                        tc.tile_pool(name="yps", bufs=4, space=bass.MemorySpace.PSUM)
                    )
                    y_pool = ph.enter_context(tc.tile_pool(name="yp", bufs=5))
    
                    TQC = mlp_chunk // 512
                    NTC = mlp_chunk // P
                    for mc in range(MC):
                        q0 = mc * mlp_chunk
                        gT = g_pool.tile([P, HT, mlp_chunk], BF16, tag="g")
                        assert TQC == 1
                        # bias-free path merges gelu over hidden-tile pairs
                        # (per-partition bias differs across the pair, so the
                        # merged op is only valid with zero b1)
                        GHT = 1 if with_bias else 2
                        for ht in range(0, HT, GHT):
                            psf = f_ps.tile(
                                [P, 512 * GHT], F32, tag="f", name=f"psf_{mc}_{ht}"
                            )
                            for sub in range(GHT):
                                w1_t = w1_pool.tile([P, DT, P], BF16, tag="w1")
                                nc.gpsimd.dma_start(
                                    out=w1_t[:],
                                    in_=dw1[ht + sub].rearrange("p (dt o) -> p dt o", o=P),
                                )
                                for dt in range(DT):
                                    nc.tensor.matmul(
                                        psf[:, sub * 512 : (sub + 1) * 512],
                                        lhsT=w1_t[:, dt, :],
                                        rhs=h2T[:, dt, q0 : q0 + 512],
                                        start=(dt == 0),
                                        stop=(dt == DT - 1),
                                    )
                            nc.scalar.activation(
                                out=gT[:, ht : ht + GHT, :].rearrange(
                                    "p a b -> p (a b)"
                                ),
                                in_=psf[:],
                                func=AF.Gelu if gelu else AF.Identity,
                                bias=b1_pp[:, ht : ht + 1] if with_bias else 0.0,
                                scale=1.0,
                            )
                        y_ts = [
                            y_pool.tile([P, DIM], F32, tag="yt", name=f"yt_{mc}_{i}")
                            for i in range(NTC)
                        ]
                        for c0, cw in ((0, 512), (512, 256)):
                            psy = [
                                y_ps.tile([P, 512], F32, tag="y", name=f"psy_{mc}_{c0}_{i}")
                                for i in range(NTC)
                            ]
                            for ht in range(HT):
                                w2_t = w2_pool.tile([P, 512], BF16, tag="w2")
                                nc.sync.dma_start(
                                    out=w2_t[:, :cw],
                                    in_=dw2[ht * P : (ht + 1) * P, c0 : c0 + cw],
                                )
                                for t in range(NTC):
                                    nc.tensor.matmul(
                                        psy[t][:, :cw],
                                        lhsT=gT[:, ht, t * P : (t + 1) * P],
                                        rhs=w2_t[:, :cw],
                                        start=(ht == 0),
                                        stop=(not with_bias and ht == HT - 1),
                                    )
                            if with_bias:
                                for t in range(NTC):
                                    nc.tensor.matmul(
                                        psy[t][:, :cw],
                                        lhsT=r(ones_col[:, :]),
                                        rhs=r(b2_row[:, c0 : c0 + cw]),
                                        start=False,
                                        stop=True,
                                    )
                            for t in range(NTC):
                                tg = mc * NTC + t
                                nc.vector.tensor_tensor(
                                    out=y_ts[t][:, c0 : c0 + cw],
                                    in0=psy[t][:, :cw],
                                    in1=x2_sb[:, tg, c0 : c0 + cw],
                                    op=ALU.add,
                                )
                        for t in range(NTC):
                            nc.sync.dma_start(out=yv[:, mc * NTC + t, :], in_=y_ts[t][:])


        else:
            with ExitStack() as s_dummy:
                dpool = s_dummy.enter_context(tc.tile_pool(name="dumy", bufs=1))
                dt_ = dpool.tile([P, DIM], F32)
                nc.vector.memset(dt_[:], 0.0)
                for t in range(NQT):
                    nc.sync.dma_start(out=yv[:, t, :], in_=dt_[:])
    nc.compile()
    return nc


# ---------------- host-side preprocessing ----------------


def make_core_inputs(inp, core, S=2048, NQ=1024):
    b, half = core // 2, core % 2
    q0 = half * NQ
    x = np.asarray(inp["x"][b], np.float32)
    xrot = np.concatenate([x[q0 : q0 + NQ], x[:q0] if q0 else x[NQ:]], axis=0)
    mask = np.asarray(inp["mask"][b, 0], np.float32)
    mq = mask[q0 : q0 + NQ]
    mrot = np.concatenate(
        [mq[:, q0 : q0 + NQ], mq[:, :q0] if q0 else mq[:, NQ:]], axis=1
    )
    maskT = np.ascontiguousarray(mrot.T).astype(ml_dtypes.bfloat16)

    g1 = np.asarray(inp["g1"], np.float32)
    be1 = np.asarray(inp["beta1"], np.float32)
    g2 = np.asarray(inp["g2"], np.float32)
    be2 = np.asarray(inp["beta2"], np.float32)
    w_qkv = np.asarray(inp["w_qkv"], np.float32)
    wqkv = w_qkv * g1[:, None]
    bqkv = np.asarray(inp["b_qkv"], np.float32) + be1 @ w_qkv
    w1f = np.asarray(inp["w1"], np.float32)
    w1 = w1f * g2[:, None]
    b1 = np.asarray(inp["b1"], np.float32) + be2 @ w1f

    e2map = np.zeros((2, P), np.float32)
    e2map[0, :DH] = 1.0
    e2map[1, DH:] = 1.0

    KTl, NQTl = S // P, NQ // P
    xpk = np.ascontiguousarray(xrot.reshape(KTl, P, DIM).transpose(1, 0, 2))
    mpk = np.ascontiguousarray(maskT.reshape(KTl, P, NQ).transpose(1, 0, 2))
    wq16 = wqkv.astype(ml_dtypes.bfloat16)
    wvr = np.ascontiguousarray(
        wq16[:, 2 * DIM :].reshape(DT, P, DIM).transpose(1, 0, 2)
    )
    wkqr = np.zeros((2 * DT, P, DT * P), ml_dtypes.bfloat16)
    for m in range(DT):
        wkqr[m] = (
            wq16[:, DIM + m * P : DIM + (m + 1) * P]
            .reshape(DT, P, P).transpose(1, 0, 2).reshape(P, DT * P)
        )
        wkqr[DT + m] = (
            wq16[:, m * P : (m + 1) * P]
            .reshape(DT, P, P).transpose(1, 0, 2).reshape(P, DT * P)
        )
    wprojr = np.ascontiguousarray(
        np.asarray(inp["w_proj"], np.float32).reshape(DT, P, DIM).transpose(1, 0, 2)
    ).astype(ml_dtypes.bfloat16)
    HTl = HID // P
    w1r = np.ascontiguousarray(
        w1.reshape(DT, P, HTl, P).transpose(2, 1, 0, 3).reshape(HTl, P, DT * P)
    ).astype(ml_dtypes.bfloat16)
    return {
        "x": xpk,
        "maskT": mpk,
        "wvr": wvr,
        "wkqr": wkqr,
        "bqkv_pp": np.ascontiguousarray(bqkv.reshape(3 * DIM // P, P).T),
        "bv_row": bqkv[2 * DIM :].reshape(1, DIM).copy(),
        "wprojr": wprojr,
        "bproj_row": np.asarray(inp["b_proj"], np.float32).reshape(1, DIM).copy(),
        "w1r": w1r,
        "b1_pp": np.ascontiguousarray(b1.reshape(HID // P, P).T),
        "w2": np.asarray(inp["w2"], np.float32).astype(ml_dtypes.bfloat16),
        "b2_row": np.asarray(inp["b2"], np.float32).reshape(1, DIM).copy(),
        "e2map": e2map,
        "ones_row": np.ones((1, P), np.float32),
        "ident_bf": np.eye(P, dtype=ml_dtypes.bfloat16),
        "ident_f": np.eye(P, dtype=np.float32),
    }


def assemble_output(results, B=4, S=2048, NQ=1024):
    y = np.zeros((B, S, DIM), np.float32)
    for core, res in enumerate(results):
        b, half = core // 2, core % 2
        yr = res["y"].reshape(P, NQ // P, DIM).transpose(1, 0, 2).reshape(NQ, DIM)
        y[b, half * NQ : (half + 1) * NQ] = yr
    return y


# ---------------- harness entry point ----------------

_NC_CACHE = {}


def _get_nc(with_bias=True):
    key = ("nc", with_bias)
    if key not in _NC_CACHE:
        _NC_CACHE[key] = build_nc(gelu=True, with_bias=with_bias)
    return _NC_CACHE[key]


def needs_bias(in_maps):
    """True unless every in-kernel bias add is provably zero (the common
    case here: the extra bias matmuls + unmerged gelu are then skipped)."""
    m = in_maps[0]
    return any(
        np.any(np.asarray(m[k], np.float32))
        for k in ("bv_row", "bproj_row", "b1_pp", "b2_row")
    )


def kernel(**inputs):
    """Full (unsharded) inputs -> full (4, 2048, 768) float32 output.

    Shards batch x query-half across the 8 NeuronCores, runs the Bass/Tile
    kernel SPMD, and reassembles the output.
    """
    from concourse.bass_utils import run_bass_kernel_spmd

    in_maps = [make_core_inputs(inputs, c) for c in range(8)]
    nc = _get_nc(with_bias=needs_bias(in_maps))
    res = run_bass_kernel_spmd(nc, in_maps, core_ids=list(range(8)))
    return assemble_output(res.results)



# revision 28
# speedup vs baseline: 1.0506x; 1.0506x over previous
"""Bass/Tile kernel builder for the pre-LN attention block (dense_transformer).

Sharding: 8 cores = 4 batches x 2 query-halves. Each core:
  - loads x for its full batch; per 128-row block: LN1, transpose -> hT
    (dim-major, bf16), V matmuls for that block (PE fills DVE/DMA gaps)
  - attention per head-pair j with the K/Q matmuls for pair j+1 interleaved
    into the kt loop (PE fills the exp/mask bubbles; Act engine is the
    phase-C floor); scores kept transposed [k, q]: no max-subtraction
    (|score| <= ~9), denominator via ones-column appended to V
  - proj + residual (row-major), LN2, MLP, residual, store y rows

Dtypes: bf16 everywhere on the matmul paths (weights incl. proj/MLP), fp32
residuals/stats/denominators. Bias matmuls skipped when biases are all zero
(with_bias=False); host folds gamma/beta into weights either way.
SPMD trick: host rotates rows so each core's own rows are always [0, NQ).

PSUM budget: A: tp1(3x1)+qkvps(2x2)=7 banks; B/C: sps(2x2)+avps(2x2)=8,
K/Q units share the sps pool slots. D: nbps+prps; E: fps+yps.
"""

import sys

sys.path.insert(0, "/opt/trn_rl_repo")

from contextlib import ExitStack

import numpy as np
import ml_dtypes

import concourse.bass as bass
import concourse.tile as tile
import concourse.mybir as mybir
from concourse import bacc

F32 = mybir.dt.float32
F32R = mybir.dt.float32r
BF16 = mybir.dt.bfloat16
AF = mybir.ActivationFunctionType
ALU = mybir.AluOpType

DIM = 768
H = 12
DH = 64
HID = 3072
SCALE = DH ** -0.5
EPS = 1e-6
P = 128
DT = DIM // P


def r(x):
    return x.bitcast(F32R)


def build_nc(S=2048, NQ=1024, mlp_chunk=512, gelu=True, repeat=1, stop_after=None, with_bias=True):
    KT = S // P
    NQT = NQ // P
    assert NQ % 512 == 0
    QC = NQ // 512
    HT = HID // P
    MC = NQ // mlp_chunk
    KCW = min(1024, S)
    QCW = min(1024, NQ)

    nc = bacc.Bacc("TRN2", target_bir_lowering=False, debug=False, num_devices=8)

    dx = nc.dram_tensor("x", [P, S // P, DIM], F32, kind="ExternalInput").ap()
    dmask = nc.dram_tensor("maskT", [P, S // P, NQ], BF16, kind="ExternalInput").ap()
    dwv = nc.dram_tensor("wvr", [P, DT, DIM], BF16, kind="ExternalInput").ap()
    dwkq = nc.dram_tensor("wkqr", [2 * DT, P, DT * P], BF16, kind="ExternalInput").ap()
    dbqkv = nc.dram_tensor("bqkv_pp", [P, 3 * DT], F32, kind="ExternalInput").ap()
    dbv = nc.dram_tensor("bv_row", [1, DIM], F32R, kind="ExternalInput").ap()
    dwproj = nc.dram_tensor("wprojr", [P, DT, DIM], BF16, kind="ExternalInput").ap()
    dbproj = nc.dram_tensor("bproj_row", [1, DIM], F32R, kind="ExternalInput").ap()
    dw1 = nc.dram_tensor("w1r", [HT, P, DT * P], BF16, kind="ExternalInput").ap()
    db1 = nc.dram_tensor("b1_pp", [P, HT], F32, kind="ExternalInput").ap()
    dw2 = nc.dram_tensor("w2", [HID, DIM], BF16, kind="ExternalInput").ap()
    db2 = nc.dram_tensor("b2_row", [1, DIM], F32R, kind="ExternalInput").ap()
    de2map = nc.dram_tensor("e2map", [2, P], F32, kind="ExternalInput").ap()
    dones = nc.dram_tensor("ones_row", [1, P], F32R, kind="ExternalInput").ap()
    dident_bf = nc.dram_tensor("ident_bf", [P, P], BF16, kind="ExternalInput").ap()
    dident_f = nc.dram_tensor("ident_f", [P, P], F32, kind="ExternalInput").ap()
    dy = nc.dram_tensor("y", [P, NQ // P, DIM], F32, kind="ExternalOutput").ap()

    xv = dx
    maskv = dmask
    yv = dy

    with nc.allow_low_precision(
        reason="fp32r matmuls + bf16 attention path validated offline"
    ), tile.TileContext(nc) as tc, ExitStack() as top:
        rep_ctx = tc.For_i(0, repeat, 1) if repeat > 1 else ExitStack()
        top.enter_context(rep_ctx)
        consts = top.enter_context(tc.tile_pool(name="consts", bufs=1))
        ident_bf = consts.tile([P, P], BF16)
        nc.sync.dma_start(out=ident_bf[:], in_=dident_bf[:])
        e2map_sb = consts.tile([2, P], F32)
        nc.gpsimd.dma_start(out=e2map_sb[:], in_=de2map[:])
        eps_t = consts.tile([P, 1], F32)
        nc.vector.memset(eps_t[:], EPS)
        bqkv_pp = consts.tile([P, 3 * DT], F32)
        nc.gpsimd.dma_start(out=bqkv_pp[:], in_=dbqkv[:])
        if with_bias:
            ones_col = consts.tile([1, P], F32)
            nc.gpsimd.dma_start(out=r(ones_col[:]), in_=dones[:])
            bv_row = consts.tile([1, DIM], F32)
            nc.gpsimd.dma_start(out=r(bv_row[:]), in_=dbv[:])
            bproj_row = consts.tile([1, DIM], F32)
            nc.gpsimd.dma_start(out=r(bproj_row[:]), in_=dbproj[:])
            b1_pp = consts.tile([P, HT], F32)
            nc.gpsimd.dma_start(out=b1_pp[:], in_=db1[:])
            b2_row = consts.tile([1, DIM], F32)
            nc.gpsimd.dma_start(out=r(b2_row[:]), in_=db2[:])

        def layer_norm_tile(stats_pool, x_ap, out_ap):
            stats = stats_pool.tile([P, 2, 6], F32, tag="lnstats")
            for sg in range(2):
                nc.vector.bn_stats(
                    out=stats[:, sg, :], in_=x_ap[:, sg * 384 : (sg + 1) * 384]
                )
            mv = stats_pool.tile([P, 2], F32, tag="lnmv")
            nc.vector.bn_aggr(out=mv[:], in_=stats[:])
            sd = stats_pool.tile([P, 1], F32, tag="lnsd")
            nc.scalar.activation(
                out=sd[:], in_=mv[:, 1:2], func=AF.Sqrt, bias=eps_t[:], scale=1.0
            )
            rstd = stats_pool.tile([P, 1], F32, tag="lnrstd")
            nc.vector.reciprocal(out=rstd[:], in_=sd[:])
            nc.vector.tensor_scalar(
                out=out_ap,
                in0=x_ap,
                scalar1=mv[:, 0:1],
                scalar2=rstd[:],
                op0=ALU.subtract,
                op1=ALU.mult,
            )

        wa_pool = top.enter_context(tc.tile_pool(name="wa", bufs=1))
        waT = wa_pool.tile([P, DT, NQ], BF16)
        recip_all = wa_pool.tile([2, H // 2, NQ], F32)

        with ExitStack() as s_kqv:
            kqv_pool = s_kqv.enter_context(tc.tile_pool(name="kqv", bufs=1))
            k_sb = kqv_pool.tile([P, DT, S], BF16)
            q_sb = kqv_pool.tile([P, DT, NQ], BF16)
            v_sb = kqv_pool.tile([P, KT, H * (DH + 1)], BF16)
            mask_sb = kqv_pool.tile([P, KT, NQ], BF16)
            nc.gpsimd.dma_start(out=mask_sb[:], in_=maskv[:])
            v4 = v_sb.rearrange("p t (h s) -> p t h s", s=DH + 1)
            nc.vector.memset(v4[:, :, :, DH : DH + 1], 1.0)

            with ExitStack() as s_ht:
                ht_pool = s_ht.enter_context(tc.tile_pool(name="htp", bufs=1))
                hT = ht_pool.tile([P, DT, S], BF16)

                # ---------- phase A: LN1 + transpose -> hT, V per block ----------
                with ExitStack() as ph:
                    wv_pool = ph.enter_context(tc.tile_pool(name="wv", bufs=1))
                    wv_sb = wv_pool.tile([P, DT, DIM], BF16)
                    nc.gpsimd.dma_start(out=wv_sb[:], in_=dwv[:])
                    xo_pool = ph.enter_context(tc.tile_pool(name="xo", bufs=4))
                    h_pool = ph.enter_context(tc.tile_pool(name="h1", bufs=4))
                    st_pool = ph.enter_context(tc.tile_pool(name="st1", bufs=6))
                    tp_pool = ph.enter_context(
                        tc.tile_pool(name="tp1", bufs=3, space=bass.MemorySpace.PSUM)
                    )
                    qkv_ps = ph.enter_context(
                        tc.tile_pool(name="qkvps", bufs=2, space=bass.MemorySpace.PSUM)
                    )

                    def v_block(t):
                        psv = qkv_ps.tile([P, DIM], F32, tag="qkvps", name=f"psv_{t}")
                        for dt in range(DT):
                            for c0, cw in ((0, 512), (512, 256)):
                                nc.tensor.matmul(
                                    psv[:, c0 : c0 + cw],
                                    lhsT=hT[:, dt, t * P : (t + 1) * P],
                                    rhs=wv_sb[:, dt, c0 : c0 + cw],
                                    start=(dt == 0),
                                    stop=(not with_bias and dt == DT - 1),
                                )
                        if with_bias:
                            for c0, cw in ((0, 512), (512, 256)):
                                nc.tensor.matmul(
                                    psv[:, c0 : c0 + cw],
                                    lhsT=r(ones_col[:, :]),
                                    rhs=r(bv_row[:, c0 : c0 + cw]),
                                    start=False,
                                    stop=True,
                                )
                        nc.vector.tensor_copy(
                            out=v4[:, t, 0:H, 0:DH],
                            in_=psv[:].rearrange("p (h s) -> p h s", s=DH),
                        )

                    INTERLEAVE_V = False
                    for t in range(KT):
                        xo = xo_pool.tile([P, DIM], F32, tag="xo")
                        nc.sync.dma_start(out=xo[:], in_=xv[:, t, :])
                        h_t = h_pool.tile([P, DIM], BF16, tag="h")
                        layer_norm_tile(st_pool, xo[:], h_t[:])
                        tp = tp_pool.tile([P, DIM], BF16, tag="tp", name=f"tp_{t}")
                        for dt in range(DT):
                            nc.tensor.transpose(
                                tp[:, dt * P : (dt + 1) * P],
                                h_t[:, dt * P : (dt + 1) * P],
                                ident_bf[:],
                            )
                        nc.vector.tensor_copy(
                            out=hT[:, :, t * P : (t + 1) * P],
                            in_=tp[:].rearrange("p (d o) -> p d o", o=P),
                        )
                        if INTERLEAVE_V and t > 0:
                            v_block(t - 1)
                    if INTERLEAVE_V:
                        v_block(KT - 1)
                    else:
                        for t in range(KT):
                            v_block(t)

                if stop_after != "ab":
                    # ---------- phase B/C: attention, K/Q(j+1) interleaved ----------
                    # Per (j, hh) pass: scores computed one kt ahead of AV so
                    # the Act engine (exp, the phase floor) always has its
                    # next input ready; K/Q matmuls for pair j+1 are sliced
                    # into 2-matmul pieces dropped between scores and AV.
                    with ExitStack() as ph:
                        wqk_pool = ph.enter_context(tc.tile_pool(name="wqk", bufs=4))
                        s_ps = ph.enter_context(
                            tc.tile_pool(name="sps", bufs=2, space=bass.MemorySpace.PSUM)
                        )
                        av_ps = ph.enter_context(
                            tc.tile_pool(name="avps", bufs=1, space=bass.MemorySpace.PSUM)
                        )
                        kq_ps = ph.enter_context(
                            tc.tile_pool(name="kqps", bufs=1, space=bass.MemorySpace.PSUM)
                        )
                        p_pool = ph.enter_context(tc.tile_pool(name="pp", bufs=5))
                        dtmp_pool = ph.enter_context(tc.tile_pool(name="dtmp", bufs=2))
                        scr_pool = ph.enter_context(tc.tile_pool(name="scr", bufs=1))

                        def load_w(m, qk):
                            w = wqk_pool.tile(
                                [P, DT, P], BF16, tag="wqk", name=f"w{qk}_{m}"
                            )
                            idx = m if qk == "k" else DT + m
                            nc.gpsimd.dma_start(
                                out=w[:],
                                in_=dwkq[idx].rearrange("p (dt o) -> p dt o", o=P),
                            )
                            return w

                        def kq_piece(m, w, qk, cp, dt, unit_box):
                            if dt == 0:
                                unit_box[0] = kq_ps.tile(
                                    [P, 1024], F32, tag="kq", name=f"{qk}u_{m}_{cp}"
                                )
                            ps = unit_box[0]
                            for ci in range(2):
                                c = cp * 2 + ci
                                nc.tensor.matmul(
                                    ps[:, ci * 512 : (ci + 1) * 512],
                                    lhsT=w[:, dt, :],
                                    rhs=hT[:, dt, c * 512 : (c + 1) * 512],
                                    start=(dt == 0),
                                    stop=(dt == DT - 1),
                                )
                            if dt == DT - 1:
                                dst, boff = (k_sb, DT) if qk == "k" else (q_sb, 0)
                                nc.vector.tensor_scalar_add(
                                    out=dst[:, m, cp * 1024 : (cp + 1) * 1024],
                                    in0=ps[:],
                                    scalar1=bqkv_pp[:, boff + m : boff + m + 1],
                                )

                        def kq_full(m, w, qk, cp):
                            box = [None]
                            for dt in range(DT):
                                kq_piece(m, w, qk, cp, dt, box)

                        wk_cur = load_w(0, "k")
                        wq_cur = load_w(0, "q")
                        kq_full(0, wk_cur, "k", 0)
                        kq_full(0, wk_cur, "k", 1)
                        kq_full(0, wq_cur, "q", 0)

                        def psb_piece(jj, c, box):
                            # normalize waT[:, jj] by its softmax denominators:
                            # e2map broadcasts the two recip rows to 128
                            # partitions (plain-fp32 matmul; fp32r would need
                            # an fp32r-rounded producer)
                            if c == 0:
                                box[0] = kq_ps.tile(
                                    [P, NQ], F32, tag="kq", name=f"psb_{jj}"
                                )
                            psb = box[0]
                            nc.tensor.matmul(
                                psb[:, c * 512 : (c + 1) * 512],
                                lhsT=e2map_sb[:, :],
                                rhs=recip_all[:, jj, c * 512 : (c + 1) * 512],
                            )
                            if c == 1:
                                nc.vector.tensor_tensor(
                                    out=waT[:, jj, :],
                                    in0=waT[:, jj, :],
                                    in1=psb[:],
                                    op=ALU.mult,
                                )

                        PIECE_SLOTS = set(range(0, 2 * KT, 2)) | {1, 3}
                        for j in range(H // 2):
                            if j + 1 < H // 2:
                                wk_cur = load_w(j + 1, "k")
                                wq_cur = load_w(j + 1, "q")
                                pieces = [
                                    ("kq", qk, cp, dt)
                                    for qk, cp in (("k", 0), ("k", 1), ("q", 0))
                                    for dt in range(DT)
                                ]
                            else:
                                pieces = [
                                    ("psb", jj, c)
                                    for jj in range(H // 2 - 1)
                                    for c in range(2)
                                ]
                            pc = 0
                            unit_box = [None]
                            den_j = dtmp_pool.tile(
                                [2, NQ], F32, tag="den", name=f"den_{j}"
                            )
                            for hh in range(2):
                                lo, hi = hh * DH, (hh + 1) * DH
                                hgl = (2 * j + hh) * (DH + 1)

                                def scores_mm(kt):
                                    pss = s_ps.tile(
                                        [P, NQ], F32, tag="s",
                                        name=f"s_{j}_{hh}_{kt}",
                                    )
                                    for c in range(QC):
                                        nc.tensor.matmul(
                                            pss[:, c * 512 : (c + 1) * 512],
                                            lhsT=k_sb[lo:hi, j, kt * P : (kt + 1) * P],
                                            rhs=q_sb[lo:hi, j, c * 512 : (c + 1) * 512],
                                            tile_position=(lo, 0),
                                        )
                                    pe_t = p_pool.tile([P, NQ], BF16, tag="pe")
                                    nc.scalar.activation(
                                        out=pe_t[:], in_=pss[:], func=AF.Exp, scale=SCALE
                                    )
                                    pt = p_pool.tile([P, NQ], BF16, tag="p")
                                    nc.vector.tensor_tensor(
                                        out=pt[:],
                                        in0=pe_t[:],
                                        in1=mask_sb[:, kt, :],
                                        op=ALU.mult,
                                    )
                                    return pt

                                av = av_ps.tile(
                                    [P, NQ], F32, tag="av", name=f"av_{j}_{hh}"
                                )
                                pt_cur = scores_mm(0)
                                for kt in range(KT):
                                    pt_next = scores_mm(kt + 1) if kt + 1 < KT else None
                                    if hh * KT + kt in PIECE_SLOTS and pc < len(pieces):
                                        piece = pieces[pc]
                                        if piece[0] == "kq":
                                            _, qk, cp, dt = piece
                                            kq_piece(
                                                j + 1,
                                                wk_cur if qk == "k" else wq_cur,
                                                qk, cp, dt, unit_box,
                                            )
                                        else:
                                            _, jj, c = piece
                                            psb_piece(jj, c, unit_box)
                                        pc += 1
                                    for c in range(QC):
                                        nc.tensor.matmul(
                                            av[0 : DH + 1, c * 512 : (c + 1) * 512],
                                            lhsT=v_sb[:, kt, hgl : hgl + DH + 1],
                                            rhs=pt_cur[:, c * 512 : (c + 1) * 512],
                                            start=(kt == 0),
                                            stop=(kt == KT - 1),
                                        )
                                    pt_cur = pt_next
                                nc.vector.tensor_copy(
                                    out=waT[hh * DH : (hh + 1) * DH, j, :],
                                    in_=av[0:DH, :],
                                )
                                dtmp = dtmp_pool.tile([P, NQ], F32, tag="dtmp")
                                nc.vector.tensor_copy(
                                    out=dtmp[DH : DH + 1, :], in_=av[DH : DH + 1, :]
                                )
                                (nc.sync if hh else nc.gpsimd).dma_start(
                                    out=den_j[hh : hh + 1, :], in_=dtmp[DH : DH + 1, :]
                                )
                            scr_j = scr_pool.tile([2, NQ], F32, tag="scr", name=f"scr_{j}")
                            nc.vector.reciprocal_approx_accurate(
                                out=recip_all[:, j, :], in_=den_j[:], scratch=scr_j[:]
                            )
                        box5 = [None]
                        psb_piece(H // 2 - 1, 0, box5)
                        psb_piece(H // 2 - 1, 1, box5)

        if stop_after is None:
            # ---------- phases D+E ----------
            with ExitStack() as s_de:
                de_pool = s_de.enter_context(tc.tile_pool(name="de", bufs=1))
                x2_sb = de_pool.tile([P, NQT, DIM], F32)
                h2T = de_pool.tile([P, DT, NQ], BF16)

                # ---------- phase D: normalize wa + proj + residual + LN2 ----------
                with ExitStack() as ph:
                    wp_pool = ph.enter_context(tc.tile_pool(name="wp", bufs=1))
                    xr_pool = ph.enter_context(tc.tile_pool(name="xr", bufs=3))
                    wproj_sb = wp_pool.tile([P, DT, DIM], BF16)
                    nc.gpsimd.dma_start(
                        out=wproj_sb[:],
                        in_=dwproj[:],
                    )
                    with ExitStack() as ph2:
                        pr_ps = ph2.enter_context(
                            tc.tile_pool(name="prps", bufs=3, space=bass.MemorySpace.PSUM)
                        )
                        for t in range(NQT):
                            xr = xr_pool.tile([P, DIM], F32, tag="xr")
                            nc.sync.dma_start(out=xr[:], in_=xv[:, t, :])
                            psp = pr_ps.tile([P, DIM], F32, tag="pr")
                            for dt in range(DT):
                                for c0, cw in ((0, 512), (512, 256)):
                                    nc.tensor.matmul(
                                        psp[:, c0 : c0 + cw],
                                        lhsT=waT[:, dt, t * P : (t + 1) * P],
                                        rhs=wproj_sb[:, dt, c0 : c0 + cw],
                                        start=(dt == 0),
                                        stop=(not with_bias and dt == DT - 1),
                                    )
                            if with_bias:
                                for c0, cw in ((0, 512), (512, 256)):
                                    nc.tensor.matmul(
                                        psp[:, c0 : c0 + cw],
                                        lhsT=r(ones_col[:, :]),
                                        rhs=r(bproj_row[:, c0 : c0 + cw]),
                                        start=False,
                                        stop=True,
                                    )
                            nc.vector.tensor_tensor(
                                out=x2_sb[:, t, :], in0=psp[:], in1=xr[:], op=ALU.add
                            )
                    h2_pool = ph.enter_context(tc.tile_pool(name="h2", bufs=4))
                    st2_pool = ph.enter_context(tc.tile_pool(name="st2", bufs=6))
                    tp2_pool = ph.enter_context(
                        tc.tile_pool(name="tp2", bufs=7, space=bass.MemorySpace.PSUM)
                    )
                    for tg in range(NQT // 4):
                        ps = [
                            tp2_pool.tile([P, 512], BF16, tag="tp2", name=f"tp2_{tg}_{i}")
                            for i in range(DT)
                        ]
                        for tt in range(4):
                            t = tg * 4 + tt
                            h2_t = h2_pool.tile([P, DIM], BF16, tag="h2")
                            layer_norm_tile(st2_pool, x2_sb[:, t, :], h2_t[:])
                            for dt in range(DT):
                                nc.tensor.transpose(
                                    ps[dt][:, tt * P : (tt + 1) * P],
                                    h2_t[:, dt * P : (dt + 1) * P],
                                    ident_bf[:],
                                )
                        for dt in range(DT):
                            nc.vector.tensor_copy(
                                out=h2T[:, dt, tg * 512 : (tg + 1) * 512], in_=ps[dt][:]
                            )

                # ---------- phase E: MLP ----------
                with ExitStack() as ph:
                    w1_pool = ph.enter_context(tc.tile_pool(name="w1p", bufs=4))
                    w2_pool = ph.enter_context(tc.tile_pool(name="w2p", bufs=4))
                    g_pool = ph.enter_context(tc.tile_pool(name="gp", bufs=1))
                    f_ps = ph.enter_context(
                        tc.tile_pool(name="fps", bufs=2 if not with_bias else 3, space=bass.MemorySpace.PSUM)
                    )
                    y_ps = ph.enter_context(
                        tc.tile_pool(name="yps", bufs=4, space=bass.MemorySpace.PSUM)
                    )
                    y_pool = ph.enter_context(tc.tile_pool(name="yp", bufs=5))
    
                    TQC = mlp_chunk // 512
                    NTC = mlp_chunk // P
                    for mc in range(MC):
                        q0 = mc * mlp_chunk
                        gT = g_pool.tile([P, HT, mlp_chunk], BF16, tag="g")
                        assert TQC == 1
                        # bias-free path merges gelu over hidden-tile pairs
                        # (per-partition bias differs across the pair, so the
                        # merged op is only valid with zero b1)
                        GHT = 1 if with_bias else 2
                        for ht in range(0, HT, GHT):
                            psf = f_ps.tile(
                                [P, 512 * GHT], F32, tag="f", name=f"psf_{mc}_{ht}"
                            )
                            for sub in range(GHT):
                                w1_t = w1_pool.tile([P, DT, P], BF16, tag="w1")
                                nc.gpsimd.dma_start(
                                    out=w1_t[:],
                                    in_=dw1[ht + sub].rearrange("p (dt o) -> p dt o", o=P),
                                )
                                for dt in range(DT):
                                    nc.tensor.matmul(
                                        psf[:, sub * 512 : (sub + 1) * 512],
                                        lhsT=w1_t[:, dt, :],
                                        rhs=h2T[:, dt, q0 : q0 + 512],
                                        start=(dt == 0),
                                        stop=(dt == DT - 1),
                                    )
                            nc.scalar.activation(
                                out=gT[:, ht : ht + GHT, :].rearrange(
                                    "p a b -> p (a b)"
                                ),
                                in_=psf[:],
                                func=AF.Gelu if gelu else AF.Identity,
                                bias=b1_pp[:, ht : ht + 1] if with_bias else 0.0,
                                scale=1.0,
                            )
                        y_ts = [
                            y_pool.tile([P, DIM], F32, tag="yt", name=f"yt_{mc}_{i}")
                            for i in range(NTC)
                        ]
                        for c0, cw in ((0, 512), (512, 256)):
                            psy = [
                                y_ps.tile([P, 512], F32, tag="y", name=f"psy_{mc}_{c0}_{i}")
                                for i in range(NTC)
                            ]
                            for ht in range(HT):
                                w2_t = w2_pool.tile([P, 512], BF16, tag="w2")
                                nc.sync.dma_start(
                                    out=w2_t[:, :cw],
                                    in_=dw2[ht * P : (ht + 1) * P, c0 : c0 + cw],
                                )
                                for t in range(NTC):
                                    nc.tensor.matmul(
                                        psy[t][:, :cw],
                                        lhsT=gT[:, ht, t * P : (t + 1) * P],
                                        rhs=w2_t[:, :cw],
                                        start=(ht == 0),
                                        stop=(not with_bias and ht == HT - 1),
                                    )
                            if with_bias:
                                for t in range(NTC):
                                    nc.tensor.matmul(
                                        psy[t][:, :cw],
                                        lhsT=r(ones_col[:, :]),
                                        rhs=r(b2_row[:, c0 : c0 + cw]),
                                        start=False,
                                        stop=True,
                                    )
                            for t in range(NTC):
                                tg = mc * NTC + t
                                nc.vector.tensor_tensor(
                                    out=y_ts[t][:, c0 : c0 + cw],
                                    in0=psy[t][:, :cw],
                                    in1=x2_sb[:, tg, c0 : c0 + cw],
                                    op=ALU.add,
                                )
                        for t in range(NTC):
                            nc.sync.dma_start(out=yv[:, mc * NTC + t, :], in_=y_ts[t][:])


        else:
            with ExitStack() as s_dummy:
                dpool = s_dummy.enter_context(tc.tile_pool(name="dumy", bufs=1))
                dt_ = dpool.tile([P, DIM], F32)
                nc.vector.memset(dt_[:], 0.0)
                for t in range(NQT):
                    nc.sync.dma_start(out=yv[:, t, :], in_=dt_[:])
    nc.compile()
    return nc


# ---------------- host-side preprocessing ----------------


def make_core_inputs(inp, core, S=2048, NQ=1024):
    b, half = core // 2, core % 2
    q0 = half * NQ
    x = np.asarray(inp["x"][b], np.float32)
    xrot = np.concatenate([x[q0 : q0 + NQ], x[:q0] if q0 else x[NQ:]], axis=0)
    mask = np.asarray(inp["mask"][b, 0], np.float32)
    mq = mask[q0 : q0 + NQ]
    mrot = np.concatenate(
        [mq[:, q0 : q0 + NQ], mq[:, :q0] if q0 else mq[:, NQ:]], axis=1
    )
    maskT = np.ascontiguousarray(mrot.T).astype(ml_dtypes.bfloat16)

    g1 = np.asarray(inp["g1"], np.float32)
    be1 = np.asarray(inp["beta1"], np.float32)
    g2 = np.asarray(inp["g2"], np.float32)
    be2 = np.asarray(inp["beta2"], np.float32)
    w_qkv = np.asarray(inp["w_qkv"], np.float32)
    wqkv = w_qkv * g1[:, None]
    bqkv = np.asarray(inp["b_qkv"], np.float32) + be1 @ w_qkv
    w1f = np.asarray(inp["w1"], np.float32)
    w1 = w1f * g2[:, None]
    b1 = np.asarray(inp["b1"], np.float32) + be2 @ w1f

    e2map = np.zeros((2, P), np.float32)
    e2map[0, :DH] = 1.0
    e2map[1, DH:] = 1.0

    KTl, NQTl = S // P, NQ // P
    xpk = np.ascontiguousarray(xrot.reshape(KTl, P, DIM).transpose(1, 0, 2))
    mpk = np.ascontiguousarray(maskT.reshape(KTl, P, NQ).transpose(1, 0, 2))
    wq16 = wqkv.astype(ml_dtypes.bfloat16)
    wvr = np.ascontiguousarray(
        wq16[:, 2 * DIM :].reshape(DT, P, DIM).transpose(1, 0, 2)
    )
    wkqr = np.zeros((2 * DT, P, DT * P), ml_dtypes.bfloat16)
    for m in range(DT):
        wkqr[m] = (
            wq16[:, DIM + m * P : DIM + (m + 1) * P]
            .reshape(DT, P, P).transpose(1, 0, 2).reshape(P, DT * P)
        )
        wkqr[DT + m] = (
            wq16[:, m * P : (m + 1) * P]
            .reshape(DT, P, P).transpose(1, 0, 2).reshape(P, DT * P)
        )
    wprojr = np.ascontiguousarray(
        np.asarray(inp["w_proj"], np.float32).reshape(DT, P, DIM).transpose(1, 0, 2)
    ).astype(ml_dtypes.bfloat16)
    HTl = HID // P
    w1r = np.ascontiguousarray(
        w1.reshape(DT, P, HTl, P).transpose(2, 1, 0, 3).reshape(HTl, P, DT * P)
    ).astype(ml_dtypes.bfloat16)
    return {
        "x": xpk,
        "maskT": mpk,
        "wvr": wvr,
        "wkqr": wkqr,
        "bqkv_pp": np.ascontiguousarray(bqkv.reshape(3 * DIM // P, P).T),
        "bv_row": bqkv[2 * DIM :].reshape(1, DIM).copy(),
        "wprojr": wprojr,
        "bproj_row": np.asarray(inp["b_proj"], np.float32).reshape(1, DIM).copy(),
        "w1r": w1r,
        "b1_pp": np.ascontiguousarray(b1.reshape(HID // P, P).T),
        "w2": np.asarray(inp["w2"], np.float32).astype(ml_dtypes.bfloat16),
        "b2_row": np.asarray(inp["b2"], np.float32).reshape(1, DIM).copy(),
        "e2map": e2map,
        "ones_row": np.ones((1, P), np.float32),
        "ident_bf": np.eye(P, dtype=ml_dtypes.bfloat16),
        "ident_f": np.eye(P, dtype=np.float32),
    }


def assemble_output(results, B=4, S=2048, NQ=1024):
    y = np.zeros((B, S, DIM), np.float32)
    for core, res in enumerate(results):
        b, half = core // 2, core % 2
        yr = res["y"].reshape(P, NQ // P, DIM).transpose(1, 0, 2).reshape(NQ, DIM)
        y[b, half * NQ : (half + 1) * NQ] = yr
    return y


# ---------------- harness entry point ----------------

_NC_CACHE = {}


def _get_nc(with_bias=True):
    key = ("nc", with_bias)
    if key not in _NC_CACHE:
        _NC_CACHE[key] = build_nc(gelu=True, with_bias=with_bias)
    return _NC_CACHE[key]


def needs_bias(in_maps):
    """True unless every in-kernel bias add is provably zero (the common
    case here: the extra bias matmuls + unmerged gelu are then skipped)."""
    m = in_maps[0]
    return any(
        np.any(np.asarray(m[k], np.float32))
        for k in ("bv_row", "bproj_row", "b1_pp", "b2_row")
    )


def kernel(**inputs):
    """Full (unsharded) inputs -> full (4, 2048, 768) float32 output.

    Shards batch x query-half across the 8 NeuronCores, runs the Bass/Tile
    kernel SPMD, and reassembles the output.
    """
    from concourse.bass_utils import run_bass_kernel_spmd

    in_maps = [make_core_inputs(inputs, c) for c in range(8)]
    nc = _get_nc(with_bias=needs_bias(in_maps))
    res = run_bass_kernel_spmd(nc, in_maps, core_ids=list(range(8)))
    return assemble_output(res.results)



# revision 29
# speedup vs baseline: 1.1307x; 1.0763x over previous
"""Bass/Tile kernel builder for the pre-LN attention block (dense_transformer).

Sharding: 8 cores = 4 batches x 2 query-halves. Each core:
  - loads x for its full batch; per 128-row block: LN1, transpose -> hT
    (dim-major, bf16), V matmuls for that block (PE fills DVE/DMA gaps)
  - attention per head-pair j with the K/Q matmuls for pair j+1 interleaved
    into the kt loop (PE fills the exp/mask bubbles; Act engine is the
    phase-C floor); scores kept transposed [k, q]: no max-subtraction
    (|score| <= ~9), denominator via ones-column appended to V
  - proj + residual (row-major), LN2, MLP, residual, store y rows

Dtypes: bf16 everywhere on the matmul paths (weights incl. proj/MLP), fp32
residuals/stats/denominators. Bias matmuls skipped when biases are all zero
(with_bias=False); host folds gamma/beta into weights either way.
SPMD trick: host rotates rows so each core's own rows are always [0, NQ).

PSUM budget: A: tp1(3x1)+qkvps(2x2)=7 banks; B/C: sps(2x2)+avps(2x2)=8,
K/Q units share the sps pool slots. D: nbps+prps; E: fps+yps.
"""

import sys

sys.path.insert(0, "/opt/trn_rl_repo")

from contextlib import ExitStack

import numpy as np
import ml_dtypes

import concourse.bass as bass
import concourse.tile as tile
import concourse.mybir as mybir
from concourse import bacc

F32 = mybir.dt.float32
F32R = mybir.dt.float32r
BF16 = mybir.dt.bfloat16
AF = mybir.ActivationFunctionType
ALU = mybir.AluOpType

DIM = 768
H = 12
DH = 64
HID = 3072
SCALE = DH ** -0.5
EPS = 1e-6
P = 128
DT = DIM // P


def r(x):
    return x.bitcast(F32R)


def build_nc(S=2048, NQ=1024, mlp_chunk=512, gelu=True, repeat=1, stop_after=None, with_bias=True):
    KT = S // P
    NQT = NQ // P
    assert NQ % 512 == 0
    QC = NQ // 512
    HT = HID // P
    MC = NQ // mlp_chunk
    KCW = min(1024, S)
    QCW = min(1024, NQ)

    nc = bacc.Bacc("TRN2", target_bir_lowering=False, debug=False, num_devices=8)

    dx = nc.dram_tensor("x", [P, S // P, DIM], F32, kind="ExternalInput").ap()
    dmask = nc.dram_tensor("maskT", [P, S // P, NQ], BF16, kind="ExternalInput").ap()
    dwv = nc.dram_tensor("wvr", [P, DT, DIM], BF16, kind="ExternalInput").ap()
    dwkq = nc.dram_tensor("wkqr", [2 * DT, P, DT * P], BF16, kind="ExternalInput").ap()
    dbqkv = nc.dram_tensor("bqkv_pp", [P, 3 * DT], F32, kind="ExternalInput").ap()
    dbv = nc.dram_tensor("bv_row", [1, DIM], F32R, kind="ExternalInput").ap()
    dwproj = nc.dram_tensor("wprojr", [P, DT, DIM], BF16, kind="ExternalInput").ap()
    dbproj = nc.dram_tensor("bproj_row", [1, DIM], F32R, kind="ExternalInput").ap()
    dw1 = nc.dram_tensor("w1r", [HT, P, DT * P], BF16, kind="ExternalInput").ap()
    db1 = nc.dram_tensor("b1_pp", [P, HT], F32, kind="ExternalInput").ap()
    dw2 = nc.dram_tensor("w2", [HID, DIM], BF16, kind="ExternalInput").ap()
    db2 = nc.dram_tensor("b2_row", [1, DIM], F32R, kind="ExternalInput").ap()
    de2map = nc.dram_tensor("e2map", [2, P], F32, kind="ExternalInput").ap()
    dones = nc.dram_tensor("ones_row", [1, P], F32R, kind="ExternalInput").ap()
    dident_bf = nc.dram_tensor("ident_bf", [P, P], BF16, kind="ExternalInput").ap()
    dident_f = nc.dram_tensor("ident_f", [P, P], F32, kind="ExternalInput").ap()
    dy = nc.dram_tensor("y", [P, NQ // P, DIM], F32, kind="ExternalOutput").ap()

    xv = dx
    maskv = dmask
    yv = dy

    with nc.allow_low_precision(
        reason="fp32r matmuls + bf16 attention path validated offline"
    ), tile.TileContext(nc) as tc, ExitStack() as top:
        rep_ctx = tc.For_i(0, repeat, 1) if repeat > 1 else ExitStack()
        top.enter_context(rep_ctx)
        consts = top.enter_context(tc.tile_pool(name="consts", bufs=1))
        ident_bf = consts.tile([P, P], BF16)
        nc.sync.dma_start(out=ident_bf[:], in_=dident_bf[:])
        e2map_sb = consts.tile([2, P], F32)
        nc.gpsimd.dma_start(out=e2map_sb[:], in_=de2map[:])
        eps_t = consts.tile([P, 1], F32)
        nc.vector.memset(eps_t[:], EPS)
        bqkv_pp = consts.tile([P, 3 * DT], F32)
        nc.gpsimd.dma_start(out=bqkv_pp[:], in_=dbqkv[:])
        if with_bias:
            ones_col = consts.tile([1, P], F32)
            nc.gpsimd.dma_start(out=r(ones_col[:]), in_=dones[:])
            bv_row = consts.tile([1, DIM], F32)
            nc.gpsimd.dma_start(out=r(bv_row[:]), in_=dbv[:])
            bproj_row = consts.tile([1, DIM], F32)
            nc.gpsimd.dma_start(out=r(bproj_row[:]), in_=dbproj[:])
            b1_pp = consts.tile([P, HT], F32)
            nc.gpsimd.dma_start(out=b1_pp[:], in_=db1[:])
            b2_row = consts.tile([1, DIM], F32)
            nc.gpsimd.dma_start(out=r(b2_row[:]), in_=db2[:])

        def layer_norm_tile(stats_pool, x_ap, out_ap):
            stats = stats_pool.tile([P, 2, 6], F32, tag="lnstats")
            for sg in range(2):
                nc.vector.bn_stats(
                    out=stats[:, sg, :], in_=x_ap[:, sg * 384 : (sg + 1) * 384]
                )
            mv = stats_pool.tile([P, 2], F32, tag="lnmv")
            nc.vector.bn_aggr(out=mv[:], in_=stats[:])
            sd = stats_pool.tile([P, 1], F32, tag="lnsd")
            nc.scalar.activation(
                out=sd[:], in_=mv[:, 1:2], func=AF.Sqrt, bias=eps_t[:], scale=1.0
            )
            rstd = stats_pool.tile([P, 1], F32, tag="lnrstd")
            nc.vector.reciprocal(out=rstd[:], in_=sd[:])
            nc.vector.tensor_scalar(
                out=out_ap,
                in0=x_ap,
                scalar1=mv[:, 0:1],
                scalar2=rstd[:],
                op0=ALU.subtract,
                op1=ALU.mult,
            )

        wa_pool = top.enter_context(tc.tile_pool(name="wa", bufs=1))
        waT = wa_pool.tile([P, DT, NQ], BF16)
        recip_all = wa_pool.tile([2, H // 2, NQ], F32)

        with ExitStack() as s_kqv:
            kqv_pool = s_kqv.enter_context(tc.tile_pool(name="kqv", bufs=1))
            k_sb = kqv_pool.tile([P, DT, S], BF16)
            q_sb = kqv_pool.tile([P, DT, NQ], BF16)
            v_sb = kqv_pool.tile([P, KT, H * (DH + 1)], BF16)
            mask_sb = kqv_pool.tile([P, KT, NQ], BF16)
            nc.gpsimd.dma_start(out=mask_sb[:], in_=maskv[:])
            v4 = v_sb.rearrange("p t (h s) -> p t h s", s=DH + 1)
            nc.vector.memset(v4[:, :, :, DH : DH + 1], 1.0)

            with ExitStack() as s_ht:
                ht_pool = s_ht.enter_context(tc.tile_pool(name="htp", bufs=1))
                hT = ht_pool.tile([P, DT, S], BF16)

                # ---------- phase A: LN1 + transpose -> hT, V per block ----------
                with ExitStack() as ph:
                    wv_pool = ph.enter_context(tc.tile_pool(name="wv", bufs=1))
                    wv_sb = wv_pool.tile([P, DT, DIM], BF16)
                    nc.gpsimd.dma_start(out=wv_sb[:], in_=dwv[:])
                    xo_pool = ph.enter_context(tc.tile_pool(name="xo", bufs=4))
                    h_pool = ph.enter_context(tc.tile_pool(name="h1", bufs=4))
                    st_pool = ph.enter_context(tc.tile_pool(name="st1", bufs=6))
                    tp_pool = ph.enter_context(
                        tc.tile_pool(name="tp1", bufs=3, space=bass.MemorySpace.PSUM)
                    )
                    qkv_ps = ph.enter_context(
                        tc.tile_pool(name="qkvps", bufs=2, space=bass.MemorySpace.PSUM)
                    )

                    def v_block(t):
                        psv = qkv_ps.tile([P, DIM], F32, tag="qkvps", name=f"psv_{t}")
                        for dt in range(DT):
                            for c0, cw in ((0, 512), (512, 256)):
                                nc.tensor.matmul(
                                    psv[:, c0 : c0 + cw],
                                    lhsT=hT[:, dt, t * P : (t + 1) * P],
                                    rhs=wv_sb[:, dt, c0 : c0 + cw],
                                    start=(dt == 0),
                                    stop=(not with_bias and dt == DT - 1),
                                )
                        if with_bias:
                            for c0, cw in ((0, 512), (512, 256)):
                                nc.tensor.matmul(
                                    psv[:, c0 : c0 + cw],
                                    lhsT=r(ones_col[:, :]),
                                    rhs=r(bv_row[:, c0 : c0 + cw]),
                                    start=False,
                                    stop=True,
                                )
                        nc.vector.tensor_copy(
                            out=v4[:, t, 0:H, 0:DH],
                            in_=psv[:].rearrange("p (h s) -> p h s", s=DH),
                        )

                    for t in range(KT):
                        xo = xo_pool.tile([P, DIM], F32, tag="xo")
                        nc.sync.dma_start(out=xo[:], in_=xv[:, t, :])
                        h_t = h_pool.tile([P, DIM], BF16, tag="h")
                        layer_norm_tile(st_pool, xo[:], h_t[:])
                        tp = tp_pool.tile([P, DIM], BF16, tag="tp", name=f"tp_{t}")
                        for dt in range(DT):
                            nc.tensor.transpose(
                                tp[:, dt * P : (dt + 1) * P],
                                h_t[:, dt * P : (dt + 1) * P],
                                ident_bf[:],
                            )
                        nc.vector.tensor_copy(
                            out=hT[:, :, t * P : (t + 1) * P],
                            in_=tp[:].rearrange("p (d o) -> p d o", o=P),
                        )
                        if t > 0:
                            v_block(t - 1)
                    v_block(KT - 1)

                if stop_after != "ab":
                    # ---------- phase B/C: attention, K/Q(j+1) interleaved ----------
                    # Per (j, hh) pass: scores computed one kt ahead of AV so
                    # the Act engine (exp, the phase floor) always has its
                    # next input ready; K/Q matmuls for pair j+1 are sliced
                    # into 2-matmul pieces dropped between scores and AV.
                    with ExitStack() as ph:
                        wqk_pool = ph.enter_context(tc.tile_pool(name="wqk", bufs=4))
                        s_ps = ph.enter_context(
                            tc.tile_pool(name="sps", bufs=2, space=bass.MemorySpace.PSUM)
                        )
                        av_ps = ph.enter_context(
                            tc.tile_pool(name="avps", bufs=1, space=bass.MemorySpace.PSUM)
                        )
                        kq_ps = ph.enter_context(
                            tc.tile_pool(name="kqps", bufs=1, space=bass.MemorySpace.PSUM)
                        )
                        p_pool = ph.enter_context(tc.tile_pool(name="pp", bufs=5))
                        dtmp_pool = ph.enter_context(tc.tile_pool(name="dtmp", bufs=2))
                        scr_pool = ph.enter_context(tc.tile_pool(name="scr", bufs=1))

                        def load_w(m, qk):
                            w = wqk_pool.tile(
                                [P, DT, P], BF16, tag="wqk", name=f"w{qk}_{m}"
                            )
                            idx = m if qk == "k" else DT + m
                            nc.gpsimd.dma_start(
                                out=w[:],
                                in_=dwkq[idx].rearrange("p (dt o) -> p dt o", o=P),
                            )
                            return w

                        def kq_piece(m, w, qk, cp, dt, unit_box):
                            if dt == 0:
                                unit_box[0] = kq_ps.tile(
                                    [P, 1024], F32, tag="kq", name=f"{qk}u_{m}_{cp}"
                                )
                            ps = unit_box[0]
                            for ci in range(2):
                                c = cp * 2 + ci
                                nc.tensor.matmul(
                                    ps[:, ci * 512 : (ci + 1) * 512],
                                    lhsT=w[:, dt, :],
                                    rhs=hT[:, dt, c * 512 : (c + 1) * 512],
                                    start=(dt == 0),
                                    stop=(dt == DT - 1),
                                )
                            if dt == DT - 1:
                                dst, boff = (k_sb, DT) if qk == "k" else (q_sb, 0)
                                nc.vector.tensor_scalar_add(
                                    out=dst[:, m, cp * 1024 : (cp + 1) * 1024],
                                    in0=ps[:],
                                    scalar1=bqkv_pp[:, boff + m : boff + m + 1],
                                )

                        def kq_full(m, w, qk, cp):
                            box = [None]
                            for dt in range(DT):
                                kq_piece(m, w, qk, cp, dt, box)

                        wk_cur = load_w(0, "k")
                        wq_cur = load_w(0, "q")
                        kq_full(0, wk_cur, "k", 0)
                        kq_full(0, wk_cur, "k", 1)
                        kq_full(0, wq_cur, "q", 0)

                        def psb_piece(jj, c, box):
                            # normalize waT[:, jj] by its softmax denominators:
                            # e2map broadcasts the two recip rows to 128
                            # partitions (plain-fp32 matmul; fp32r would need
                            # an fp32r-rounded producer)
                            if c == 0:
                                box[0] = kq_ps.tile(
                                    [P, NQ], F32, tag="kq", name=f"psb_{jj}"
                                )
                            psb = box[0]
                            nc.tensor.matmul(
                                psb[:, c * 512 : (c + 1) * 512],
                                lhsT=e2map_sb[:, :],
                                rhs=recip_all[:, jj, c * 512 : (c + 1) * 512],
                            )
                            if c == 1:
                                nc.vector.tensor_tensor(
                                    out=waT[:, jj, :],
                                    in0=waT[:, jj, :],
                                    in1=psb[:],
                                    op=ALU.mult,
                                )

                        PIECE_SLOTS = set(range(0, 2 * KT, 2)) | {1, 3}
                        for j in range(H // 2):
                            if j + 1 < H // 2:
                                wk_cur = load_w(j + 1, "k")
                                wq_cur = load_w(j + 1, "q")
                                pieces = [
                                    ("kq", qk, cp, dt)
                                    for qk, cp in (("k", 0), ("k", 1), ("q", 0))
                                    for dt in range(DT)
                                ]
                            else:
                                pieces = [
                                    ("psb", jj, c)
                                    for jj in range(H // 2 - 1)
                                    for c in range(2)
                                ]
                            pc = 0
                            unit_box = [None]
                            den_j = dtmp_pool.tile(
                                [2, NQ], F32, tag="den", name=f"den_{j}"
                            )
                            for hh in range(2):
                                lo, hi = hh * DH, (hh + 1) * DH
                                hgl = (2 * j + hh) * (DH + 1)

                                def scores_mm(kt):
                                    pss = s_ps.tile(
                                        [P, NQ], F32, tag="s",
                                        name=f"s_{j}_{hh}_{kt}",
                                    )
                                    for c in range(QC):
                                        nc.tensor.matmul(
                                            pss[:, c * 512 : (c + 1) * 512],
                                            lhsT=k_sb[lo:hi, j, kt * P : (kt + 1) * P],
                                            rhs=q_sb[lo:hi, j, c * 512 : (c + 1) * 512],
                                            tile_position=(lo, 0),
                                        )
                                    pe_t = p_pool.tile([P, NQ], BF16, tag="pe")
                                    nc.scalar.activation(
                                        out=pe_t[:], in_=pss[:], func=AF.Exp, scale=SCALE
                                    )
                                    pt = p_pool.tile([P, NQ], BF16, tag="p")
                                    nc.vector.tensor_tensor(
                                        out=pt[:],
                                        in0=pe_t[:],
                                        in1=mask_sb[:, kt, :],
                                        op=ALU.mult,
                                    )
                                    return pt

                                av = av_ps.tile(
                                    [P, NQ], F32, tag="av", name=f"av_{j}_{hh}"
                                )
                                pt_cur = scores_mm(0)
                                for kt in range(KT):
                                    pt_next = scores_mm(kt + 1) if kt + 1 < KT else None
                                    if hh * KT + kt in PIECE_SLOTS and pc < len(pieces):
                                        piece = pieces[pc]
                                        if piece[0] == "kq":
                                            _, qk, cp, dt = piece
                                            kq_piece(
                                                j + 1,
                                                wk_cur if qk == "k" else wq_cur,
                                                qk, cp, dt, unit_box,
                                            )
                                        else:
                                            _, jj, c = piece
                                            psb_piece(jj, c, unit_box)
                                        pc += 1
                                    for c in range(QC):
                                        nc.tensor.matmul(
                                            av[0 : DH + 1, c * 512 : (c + 1) * 512],
                                            lhsT=v_sb[:, kt, hgl : hgl + DH + 1],
                                            rhs=pt_cur[:, c * 512 : (c + 1) * 512],
                                            start=(kt == 0),
                                            stop=(kt == KT - 1),
                                        )
                                    pt_cur = pt_next
                                nc.vector.tensor_copy(
                                    out=waT[hh * DH : (hh + 1) * DH, j, :],
                                    in_=av[0:DH, :],
                                )
                                dtmp = dtmp_pool.tile([P, NQ], F32, tag="dtmp")
                                nc.vector.tensor_copy(
                                    out=dtmp[DH : DH + 1, :], in_=av[DH : DH + 1, :]
                                )
                                (nc.sync if hh else nc.gpsimd).dma_start(
                                    out=den_j[hh : hh + 1, :], in_=dtmp[DH : DH + 1, :]
                                )
                            scr_j = scr_pool.tile([2, NQ], F32, tag="scr", name=f"scr_{j}")
                            nc.vector.reciprocal_approx_accurate(
                                out=recip_all[:, j, :], in_=den_j[:], scratch=scr_j[:]
                            )
                        box5 = [None]
                        psb_piece(H // 2 - 1, 0, box5)
                        psb_piece(H // 2 - 1, 1, box5)

        if stop_after is None:
            # ---------- phases D+E ----------
            with ExitStack() as s_de:
                de_pool = s_de.enter_context(tc.tile_pool(name="de", bufs=1))
                x2_sb = de_pool.tile([P, NQT, DIM], F32)
                h2T = de_pool.tile([P, DT, NQ], BF16)

                # ---------- phase D: normalize wa + proj + residual + LN2 ----------
                with ExitStack() as ph:
                    wp_pool = ph.enter_context(tc.tile_pool(name="wp", bufs=1))
                    xr_pool = ph.enter_context(tc.tile_pool(name="xr", bufs=3))
                    wproj_sb = wp_pool.tile([P, DT, DIM], BF16)
                    nc.gpsimd.dma_start(
                        out=wproj_sb[:],
                        in_=dwproj[:],
                    )
                    with ExitStack() as ph2:
                        pr_ps = ph2.enter_context(
                            tc.tile_pool(name="prps", bufs=3, space=bass.MemorySpace.PSUM)
                        )
                        for t in range(NQT):
                            xr = xr_pool.tile([P, DIM], F32, tag="xr")
                            nc.sync.dma_start(out=xr[:], in_=xv[:, t, :])
                            psp = pr_ps.tile([P, DIM], F32, tag="pr")
                            for dt in range(DT):
                                for c0, cw in ((0, 512), (512, 256)):
                                    nc.tensor.matmul(
                                        psp[:, c0 : c0 + cw],
                                        lhsT=waT[:, dt, t * P : (t + 1) * P],
                                        rhs=wproj_sb[:, dt, c0 : c0 + cw],
                                        start=(dt == 0),
                                        stop=(not with_bias and dt == DT - 1),
                                    )
                            if with_bias:
                                for c0, cw in ((0, 512), (512, 256)):
                                    nc.tensor.matmul(
                                        psp[:, c0 : c0 + cw],
                                        lhsT=r(ones_col[:, :]),
                                        rhs=r(bproj_row[:, c0 : c0 + cw]),
                                        start=False,
                                        stop=True,
                                    )
                            nc.vector.tensor_tensor(
                                out=x2_sb[:, t, :], in0=psp[:], in1=xr[:], op=ALU.add
                            )
                    h2_pool = ph.enter_context(tc.tile_pool(name="h2", bufs=4))
                    st2_pool = ph.enter_context(tc.tile_pool(name="st2", bufs=6))
                    tp2_pool = ph.enter_context(
                        tc.tile_pool(name="tp2", bufs=7, space=bass.MemorySpace.PSUM)
                    )
                    for tg in range(NQT // 4):
                        ps = [
                            tp2_pool.tile([P, 512], BF16, tag="tp2", name=f"tp2_{tg}_{i}")
                            for i in range(DT)
                        ]
                        for tt in range(4):
                            t = tg * 4 + tt
                            h2_t = h2_pool.tile([P, DIM], BF16, tag="h2")
                            layer_norm_tile(st2_pool, x2_sb[:, t, :], h2_t[:])
                            for dt in range(DT):
                                nc.tensor.transpose(
                                    ps[dt][:, tt * P : (tt + 1) * P],
                                    h2_t[:, dt * P : (dt + 1) * P],
                                    ident_bf[:],
                                )
                        for dt in range(DT):
                            nc.vector.tensor_copy(
                                out=h2T[:, dt, tg * 512 : (tg + 1) * 512], in_=ps[dt][:]
                            )

                # ---------- phase E: MLP ----------
                with ExitStack() as ph:
                    w1_pool = ph.enter_context(tc.tile_pool(name="w1p", bufs=4))
                    w2_pool = ph.enter_context(tc.tile_pool(name="w2p", bufs=4))
                    g_pool = ph.enter_context(tc.tile_pool(name="gp", bufs=1))
                    f_ps = ph.enter_context(
                        tc.tile_pool(name="fps", bufs=2 if not with_bias else 3, space=bass.MemorySpace.PSUM)
                    )
                    y_ps = ph.enter_context(
                        tc.tile_pool(name="yps", bufs=4, space=bass.MemorySpace.PSUM)
                    )
                    y_pool = ph.enter_context(tc.tile_pool(name="yp", bufs=5))
    
                    TQC = mlp_chunk // 512
                    NTC = mlp_chunk // P
                    for mc in range(MC):
                        q0 = mc * mlp_chunk
                        gT = g_pool.tile([P, HT, mlp_chunk], BF16, tag="g")
                        assert TQC == 1
                        # bias-free path merges gelu over hidden-tile pairs
                        # (per-partition bias differs across the pair, so the
                        # merged op is only valid with zero b1)
                        GHT = 1 if with_bias else 2
                        for ht in range(0, HT, GHT):
                            psf = f_ps.tile(
                                [P, 512 * GHT], F32, tag="f", name=f"psf_{mc}_{ht}"
                            )
                            for sub in range(GHT):
                                w1_t = w1_pool.tile([P, DT, P], BF16, tag="w1")
                                nc.gpsimd.dma_start(
                                    out=w1_t[:],
                                    in_=dw1[ht + sub].rearrange("p (dt o) -> p dt o", o=P),
                                )
                                for dt in range(DT):
                                    nc.tensor.matmul(
                                        psf[:, sub * 512 : (sub + 1) * 512],
                                        lhsT=w1_t[:, dt, :],
                                        rhs=h2T[:, dt, q0 : q0 + 512],
                                        start=(dt == 0),
                                        stop=(dt == DT - 1),
                                    )
                            nc.scalar.activation(
                                out=gT[:, ht : ht + GHT, :].rearrange(
                                    "p a b -> p (a b)"
                                ),
                                in_=psf[:],
                                func=AF.Gelu if gelu else AF.Identity,
                                bias=b1_pp[:, ht : ht + 1] if with_bias else 0.0,
                                scale=1.0,
                            )
                        y_ts = [
                            y_pool.tile([P, DIM], F32, tag="yt", name=f"yt_{mc}_{i}")
                            for i in range(NTC)
                        ]
                        for c0, cw in ((0, 512), (512, 256)):
                            psy = [
                                y_ps.tile([P, 512], F32, tag="y", name=f"psy_{mc}_{c0}_{i}")
                                for i in range(NTC)
                            ]
                            for ht in range(HT):
                                w2_t = w2_pool.tile([P, 512], BF16, tag="w2")
                                nc.sync.dma_start(
                                    out=w2_t[:, :cw],
                                    in_=dw2[ht * P : (ht + 1) * P, c0 : c0 + cw],
                                )
                                for t in range(NTC):
                                    nc.tensor.matmul(
                                        psy[t][:, :cw],
                                        lhsT=gT[:, ht, t * P : (t + 1) * P],
                                        rhs=w2_t[:, :cw],
                                        start=(ht == 0),
                                        stop=(not with_bias and ht == HT - 1),
                                    )
                            if with_bias:
                                for t in range(NTC):
                                    nc.tensor.matmul(
                                        psy[t][:, :cw],
                                        lhsT=r(ones_col[:, :]),
                                        rhs=r(b2_row[:, c0 : c0 + cw]),
                                        start=False,
                                        stop=True,
                                    )
                            for t in range(NTC):
                                tg = mc * NTC + t
                                nc.vector.tensor_tensor(
                                    out=y_ts[t][:, c0 : c0 + cw],
                                    in0=psy[t][:, :cw],
                                    in1=x2_sb[:, tg, c0 : c0 + cw],
                                    op=ALU.add,
                                )
                        for t in range(NTC):
                            nc.sync.dma_start(out=yv[:, mc * NTC + t, :], in_=y_ts[t][:])


        else:
            with ExitStack() as s_dummy:
                dpool = s_dummy.enter_context(tc.tile_pool(name="dumy", bufs=1))
                dt_ = dpool.tile([P, DIM], F32)
                nc.vector.memset(dt_[:], 0.0)
                for t in range(NQT):
                    nc.sync.dma_start(out=yv[:, t, :], in_=dt_[:])
    nc.compile()
    return nc


# ---------------- host-side preprocessing ----------------


def make_core_inputs(inp, core, S=2048, NQ=1024):
    b, half = core // 2, core % 2
    q0 = half * NQ
    x = np.asarray(inp["x"][b], np.float32)
    xrot = np.concatenate([x[q0 : q0 + NQ], x[:q0] if q0 else x[NQ:]], axis=0)
    mask = np.asarray(inp["mask"][b, 0], np.float32)
    mq = mask[q0 : q0 + NQ]
    mrot = np.concatenate(
        [mq[:, q0 : q0 + NQ], mq[:, :q0] if q0 else mq[:, NQ:]], axis=1
    )
    maskT = np.ascontiguousarray(mrot.T).astype(ml_dtypes.bfloat16)

    g1 = np.asarray(inp["g1"], np.float32)
    be1 = np.asarray(inp["beta1"], np.float32)
    g2 = np.asarray(inp["g2"], np.float32)
    be2 = np.asarray(inp["beta2"], np.float32)
    w_qkv = np.asarray(inp["w_qkv"], np.float32)
    wqkv = w_qkv * g1[:, None]
    bqkv = np.asarray(inp["b_qkv"], np.float32) + be1 @ w_qkv
    w1f = np.asarray(inp["w1"], np.float32)
    w1 = w1f * g2[:, None]
    b1 = np.asarray(inp["b1"], np.float32) + be2 @ w1f

    e2map = np.zeros((2, P), np.float32)
    e2map[0, :DH] = 1.0
    e2map[1, DH:] = 1.0

    KTl, NQTl = S // P, NQ // P
    xpk = np.ascontiguousarray(xrot.reshape(KTl, P, DIM).transpose(1, 0, 2))
    mpk = np.ascontiguousarray(maskT.reshape(KTl, P, NQ).transpose(1, 0, 2))
    wq16 = wqkv.astype(ml_dtypes.bfloat16)
    wvr = np.ascontiguousarray(
        wq16[:, 2 * DIM :].reshape(DT, P, DIM).transpose(1, 0, 2)
    )
    wkqr = np.zeros((2 * DT, P, DT * P), ml_dtypes.bfloat16)
    for m in range(DT):
        wkqr[m] = (
            wq16[:, DIM + m * P : DIM + (m + 1) * P]
            .reshape(DT, P, P).transpose(1, 0, 2).reshape(P, DT * P)
        )
        wkqr[DT + m] = (
            wq16[:, m * P : (m + 1) * P]
            .reshape(DT, P, P).transpose(1, 0, 2).reshape(P, DT * P)
        )
    wprojr = np.ascontiguousarray(
        np.asarray(inp["w_proj"], np.float32).reshape(DT, P, DIM).transpose(1, 0, 2)
    ).astype(ml_dtypes.bfloat16)
    HTl = HID // P
    w1r = np.ascontiguousarray(
        w1.reshape(DT, P, HTl, P).transpose(2, 1, 0, 3).reshape(HTl, P, DT * P)
    ).astype(ml_dtypes.bfloat16)
    return {
        "x": xpk,
        "maskT": mpk,
        "wvr": wvr,
        "wkqr": wkqr,
        "bqkv_pp": np.ascontiguousarray(bqkv.reshape(3 * DIM // P, P).T),
        "bv_row": bqkv[2 * DIM :].reshape(1, DIM).copy(),
        "wprojr": wprojr,
        "bproj_row": np.asarray(inp["b_proj"], np.float32).reshape(1, DIM).copy(),
        "w1r": w1r,
        "b1_pp": np.ascontiguousarray(b1.reshape(HID // P, P).T),
        "w2": np.asarray(inp["w2"], np.float32).astype(ml_dtypes.bfloat16),
        "b2_row": np.asarray(inp["b2"], np.float32).reshape(1, DIM).copy(),
        "e2map": e2map,
        "ones_row": np.ones((1, P), np.float32),
        "ident_bf": np.eye(P, dtype=ml_dtypes.bfloat16),
        "ident_f": np.eye(P, dtype=np.float32),
    }


def assemble_output(results, B=4, S=2048, NQ=1024):
    y = np.zeros((B, S, DIM), np.float32)
    for core, res in enumerate(results):
        b, half = core // 2, core % 2
        yr = res["y"].reshape(P, NQ // P, DIM).transpose(1, 0, 2).reshape(NQ, DIM)
        y[b, half * NQ : (half + 1) * NQ] = yr
    return y


# ---------------- harness entry point ----------------

_NC_CACHE = {}


def _get_nc(with_bias=True):
    key = ("nc", with_bias)
    if key not in _NC_CACHE:
        _NC_CACHE[key] = build_nc(gelu=True, with_bias=with_bias)
    return _NC_CACHE[key]


def needs_bias(in_maps):
    """True unless every in-kernel bias add is provably zero (the common
    case here: the extra bias matmuls + unmerged gelu are then skipped)."""
    m = in_maps[0]
    return any(
        np.any(np.asarray(m[k], np.float32))
        for k in ("bv_row", "bproj_row", "b1_pp", "b2_row")
    )


def kernel(**inputs):
    """Full (unsharded) inputs -> full (4, 2048, 768) float32 output.

    Shards batch x query-half across the 8 NeuronCores, runs the Bass/Tile
    kernel SPMD, and reassembles the output.
    """
    from concourse.bass_utils import run_bass_kernel_spmd

    in_maps = [make_core_inputs(inputs, c) for c in range(8)]
    nc = _get_nc(with_bias=needs_bias(in_maps))
    res = run_bass_kernel_spmd(nc, in_maps, core_ids=list(range(8)))
    return assemble_output(res.results)



# revision 40
# speedup vs baseline: 1.4167x; 1.2529x over previous
"""Bass/Tile kernel builder for the pre-LN attention block (dense_transformer).

Sharding: 8 cores = 4 batches x 2 query-halves. Each core:
  - loads x for its full batch; per 128-row block: LN1, transpose -> hT
    (dim-major, bf16), V matmuls for that block (PE fills DVE/DMA gaps)
  - attention per head-pair j with the K/Q matmuls for pair j+1 interleaved
    into the kt loop (PE fills the exp/mask bubbles; Act engine is the
    phase-C floor); scores kept transposed [k, q]: no max-subtraction
    (|score| <= ~9), denominator via ones-column appended to V
  - proj + residual (row-major), LN2, MLP, residual, store y rows

Dtypes: bf16 everywhere on the matmul paths (weights incl. proj/MLP), fp32
residuals/stats/denominators. Bias matmuls skipped when biases are all zero
(with_bias=False); host folds gamma/beta into weights either way.
SPMD trick: host rotates rows so each core's own rows are always [0, NQ).

PSUM budget: A: tp1(3x1)+qkvps(2x2)=7 banks; B/C: sps(2x2)+avps(2x2)=8,
K/Q units share the sps pool slots. D: nbps+prps; E: fps+yps.
"""

import sys

sys.path.insert(0, "/opt/trn_rl_repo")

from contextlib import ExitStack

import numpy as np
import ml_dtypes

import concourse.bass as bass
import concourse.tile as tile
import concourse.mybir as mybir
from concourse import bacc

F32 = mybir.dt.float32
F32R = mybir.dt.float32r
BF16 = mybir.dt.bfloat16
F8 = mybir.dt.float8e4
DR = mybir.MatmulPerfMode.DoubleRow
AF = mybir.ActivationFunctionType
ALU = mybir.AluOpType

DIM = 768
H = 12
DH = 64
HID = 3072
SCALE = DH ** -0.5
EPS = 1e-6
P = 128
DT = DIM // P


def r(x):
    return x.bitcast(F32R)


def build_nc(S=2048, NQ=1024, mlp_chunk=512, gelu=True, repeat=1, stop_after=None, with_bias=True):
    KT = S // P
    NQT = NQ // P
    assert NQ % 512 == 0
    QC = NQ // 512
    HT = HID // P
    MC = NQ // mlp_chunk
    KCW = min(1024, S)
    QCW = min(1024, NQ)

    nc = bacc.Bacc("TRN2", target_bir_lowering=False, debug=False, num_devices=8)

    dx = nc.dram_tensor("x", [P, S // P, DIM], F32, kind="ExternalInput").ap()
    dmask = nc.dram_tensor("maskT", [P, S // P, NQ], BF16, kind="ExternalInput").ap()
    dwv = nc.dram_tensor("wvr8", [P, DT // 2, 2, DIM], F8, kind="ExternalInput").ap()
    dwkq = nc.dram_tensor(
        "wkqr8", [2 * DT, P, DT // 2, 2, P], F8, kind="ExternalInput"
    ).ap()
    dbqkv = nc.dram_tensor("bqkv_pp", [P, 3 * DT], F32, kind="ExternalInput").ap()
    dbv = nc.dram_tensor("bv_row", [1, DIM], F32R, kind="ExternalInput").ap()
    dwproj = nc.dram_tensor("wprojr", [P, DT, DIM], BF16, kind="ExternalInput").ap()
    dbproj = nc.dram_tensor("bproj_row", [1, DIM], F32R, kind="ExternalInput").ap()
    dw1 = nc.dram_tensor("w1r", [HT, P, DT * P], BF16, kind="ExternalInput").ap()
    db1 = nc.dram_tensor("b1_pp", [P, HT], F32, kind="ExternalInput").ap()
    dw2 = nc.dram_tensor("w2", [HID, DIM], BF16, kind="ExternalInput").ap()
    db2 = nc.dram_tensor("b2_row", [1, DIM], F32R, kind="ExternalInput").ap()
    de2map = nc.dram_tensor("e2map", [2, P], F32, kind="ExternalInput").ap()
    dones = nc.dram_tensor("ones_row", [1, P], F32R, kind="ExternalInput").ap()
    dident_bf = nc.dram_tensor("ident_bf", [P, P], BF16, kind="ExternalInput").ap()
    dident_f8 = nc.dram_tensor("ident_f8", [P, P], F8, kind="ExternalInput").ap()
    dy = nc.dram_tensor("y", [P, NQ // P, DIM], F32, kind="ExternalOutput").ap()

    xv = dx
    maskv = dmask
    yv = dy

    with nc.allow_low_precision(
        reason="fp32r matmuls + bf16 attention path validated offline"
    ), tile.TileContext(nc) as tc, ExitStack() as top:
        rep_ctx = tc.For_i(0, repeat, 1) if repeat > 1 else ExitStack()
        top.enter_context(rep_ctx)
        consts = top.enter_context(tc.tile_pool(name="consts", bufs=1))
        ident_bf = consts.tile([P, P], BF16)
        nc.sync.dma_start(out=ident_bf[:], in_=dident_bf[:])
        ident_f8 = consts.tile([P, P], F8)
        nc.sync.dma_start(out=ident_f8[:], in_=dident_f8[:])
        e2map_sb = consts.tile([2, P], F32)
        nc.gpsimd.dma_start(out=e2map_sb[:], in_=de2map[:])
        eps_t = consts.tile([P, 1], F32)
        nc.vector.memset(eps_t[:], EPS)
        bqkv_pp = consts.tile([P, 3 * DT], F32)
        nc.gpsimd.dma_start(out=bqkv_pp[:], in_=dbqkv[:])
        if with_bias:
            ones_col = consts.tile([1, P], F32)
            nc.gpsimd.dma_start(out=r(ones_col[:]), in_=dones[:])
            bv_row = consts.tile([1, DIM], F32)
            nc.gpsimd.dma_start(out=r(bv_row[:]), in_=dbv[:])
            bproj_row = consts.tile([1, DIM], F32)
            nc.gpsimd.dma_start(out=r(bproj_row[:]), in_=dbproj[:])
            b1_pp = consts.tile([P, HT], F32)
            nc.gpsimd.dma_start(out=b1_pp[:], in_=db1[:])
            b2_row = consts.tile([1, DIM], F32)
            nc.gpsimd.dma_start(out=r(b2_row[:]), in_=db2[:])

        def layer_norm_tile(stats_pool, x_ap, out_ap):
            stats = stats_pool.tile([P, 2, 6], F32, tag="lnstats")
            for sg in range(2):
                nc.vector.bn_stats(
                    out=stats[:, sg, :], in_=x_ap[:, sg * 384 : (sg + 1) * 384]
                )
            mv = stats_pool.tile([P, 2], F32, tag="lnmv")
            nc.vector.bn_aggr(out=mv[:], in_=stats[:])
            sd = stats_pool.tile([P, 1], F32, tag="lnsd")
            nc.scalar.activation(
                out=sd[:], in_=mv[:, 1:2], func=AF.Sqrt, bias=eps_t[:], scale=1.0
            )
            rstd = stats_pool.tile([P, 1], F32, tag="lnrstd")
            nc.vector.reciprocal(out=rstd[:], in_=sd[:])
            nc.vector.tensor_scalar(
                out=out_ap,
                in0=x_ap,
                scalar1=mv[:, 0:1],
                scalar2=rstd[:],
                op0=ALU.subtract,
                op1=ALU.mult,
            )

        wa_pool = top.enter_context(tc.tile_pool(name="wa", bufs=1))
        waT = wa_pool.tile([P, DT, NQ], BF16)
        recip_all = wa_pool.tile([2, H // 2, NQ], F32)

        with ExitStack() as s_kqv:
            kqv_pool = s_kqv.enter_context(tc.tile_pool(name="kqv", bufs=1))
            k_sb = kqv_pool.tile([P, DT, S], BF16)
            q_sb = kqv_pool.tile([P, DT, NQ], BF16)
            v_sb = kqv_pool.tile([P, KT, H * (DH + 1)], BF16)
            mask_sb = kqv_pool.tile([P, KT, NQ], BF16)
            nc.gpsimd.dma_start(out=mask_sb[:], in_=maskv[:])
            v4 = v_sb.rearrange("p t (h s) -> p t h s", s=DH + 1)
            nc.vector.memset(v4[:, :, :, DH : DH + 1], 1.0)

            with ExitStack() as s_ht:
                ht_pool = s_ht.enter_context(tc.tile_pool(name="htp", bufs=1))
                hT = ht_pool.tile([P, DT, S], F8)

                # ---------- phase A: LN1 + transpose -> hT, V per block ----------
                with ExitStack() as ph:
                    wv_pool = ph.enter_context(tc.tile_pool(name="wv", bufs=1))
                    wv_sb = wv_pool.tile([P, DT // 2, 2, DIM], F8)
                    nc.gpsimd.dma_start(out=wv_sb[:], in_=dwv[:])
                    xo_pool = ph.enter_context(tc.tile_pool(name="xo", bufs=4))
                    h_pool = ph.enter_context(tc.tile_pool(name="h1", bufs=4))
                    st_pool = ph.enter_context(tc.tile_pool(name="st1", bufs=6))
                    tp_pool = ph.enter_context(
                        tc.tile_pool(name="tp1", bufs=3, space=bass.MemorySpace.PSUM)
                    )
                    qkv_ps = ph.enter_context(
                        tc.tile_pool(name="qkvps", bufs=2, space=bass.MemorySpace.PSUM)
                    )

                    def v_block(t):
                        psv = qkv_ps.tile([P, DIM], F32, tag="qkvps", name=f"psv_{t}")
                        for pr in range(DT // 2):
                            for c0, cw in ((0, 512), (512, 256)):
                                nc.tensor.matmul(
                                    psv[:, c0 : c0 + cw],
                                    lhsT=hT[:, 2 * pr : 2 * pr + 2, t * P : (t + 1) * P],
                                    rhs=wv_sb[:, pr, :, c0 : c0 + cw],
                                    start=(pr == 0),
                                    stop=(not with_bias and pr == DT // 2 - 1),
                                    perf_mode=DR,
                                )
                        if with_bias:
                            for c0, cw in ((0, 512), (512, 256)):
                                nc.tensor.matmul(
                                    psv[:, c0 : c0 + cw],
                                    lhsT=r(ones_col[:, :]),
                                    rhs=r(bv_row[:, c0 : c0 + cw]),
                                    start=False,
                                    stop=True,
                                )
                        nc.vector.tensor_copy(
                            out=v4[:, t, 0:H, 0:DH],
                            in_=psv[:].rearrange("p (h s) -> p h s", s=DH),
                        )

                    for t in range(KT):
                        xo = xo_pool.tile([P, DIM], F32, tag="xo")
                        nc.sync.dma_start(out=xo[:], in_=xv[:, t, :])
                        h_t = h_pool.tile([P, DIM], F8, tag="h")
                        layer_norm_tile(st_pool, xo[:], h_t[:])
                        tp = tp_pool.tile([P, DIM], F8, tag="tp", name=f"tp_{t}")
                        for dt in range(DT):
                            nc.tensor.transpose(
                                tp[:, dt * P : (dt + 1) * P],
                                h_t[:, dt * P : (dt + 1) * P],
                                ident_f8[:],
                            )
                        nc.vector.tensor_copy(
                            out=hT[:, :, t * P : (t + 1) * P],
                            in_=tp[:].rearrange("p (d o) -> p d o", o=P),
                        )
                        if t > 0:
                            v_block(t - 1)
                    v_block(KT - 1)

                if stop_after != "ab":
                    # ---------- phase B/C: attention, K/Q(j+1) interleaved ----------
                    # Per (j, hh) pass: scores computed one kt ahead of AV so
                    # the Act engine (exp, the phase floor) always has its
                    # next input ready; K/Q matmuls for pair j+1 are sliced
                    # into 2-matmul pieces dropped between scores and AV.
                    with ExitStack() as ph:
                        wqk_pool = ph.enter_context(tc.tile_pool(name="wqk", bufs=4))
                        s_ps = ph.enter_context(
                            tc.tile_pool(name="sps", bufs=2, space=bass.MemorySpace.PSUM)
                        )
                        av_ps = ph.enter_context(
                            tc.tile_pool(name="avps", bufs=1, space=bass.MemorySpace.PSUM)
                        )
                        kq_ps = ph.enter_context(
                            tc.tile_pool(name="kqps", bufs=1, space=bass.MemorySpace.PSUM)
                        )
                        p_pool = ph.enter_context(tc.tile_pool(name="pp", bufs=5))
                        dtmp_pool = ph.enter_context(tc.tile_pool(name="dtmp", bufs=2))
                        scr_pool = ph.enter_context(tc.tile_pool(name="scr", bufs=1))

                        def load_w(m, qk):
                            w = wqk_pool.tile(
                                [P, DT // 2, 2, P], F8, tag="wqk", name=f"w{qk}_{m}"
                            )
                            idx = m if qk == "k" else DT + m
                            nc.gpsimd.dma_start(out=w[:], in_=dwkq[idx])
                            return w

                        def kq_piece(m, w, qk, cp, pr, unit_box):
                            if pr == 0:
                                unit_box[0] = kq_ps.tile(
                                    [P, 1024], F32, tag="kq", name=f"{qk}u_{m}_{cp}"
                                )
                            ps = unit_box[0]
                            for ci in range(2):
                                c = cp * 2 + ci
                                nc.tensor.matmul(
                                    ps[:, ci * 512 : (ci + 1) * 512],
                                    lhsT=w[:, pr, :, :],
                                    rhs=hT[:, 2 * pr : 2 * pr + 2, c * 512 : (c + 1) * 512],
                                    start=(pr == 0),
                                    stop=(pr == DT // 2 - 1),
                                    perf_mode=DR,
                                )
                            if pr == DT // 2 - 1:
                                dst, boff = (k_sb, DT) if qk == "k" else (q_sb, 0)
                                nc.vector.tensor_scalar_add(
                                    out=dst[:, m, cp * 1024 : (cp + 1) * 1024],
                                    in0=ps[:],
                                    scalar1=bqkv_pp[:, boff + m : boff + m + 1],
                                )

                        def kq_full(m, w, qk, cp):
                            box = [None]
                            for pr in range(DT // 2):
                                kq_piece(m, w, qk, cp, pr, box)

                        wk_cur = load_w(0, "k")
                        wq_cur = load_w(0, "q")
                        kq_full(0, wk_cur, "k", 0)
                        kq_full(0, wk_cur, "k", 1)
                        kq_full(0, wq_cur, "q", 0)

                        def psb_piece(jj, c, box):
                            # normalize waT[:, jj] by its softmax denominators:
                            # e2map broadcasts the two recip rows to 128
                            # partitions (plain-fp32 matmul; fp32r would need
                            # an fp32r-rounded producer)
                            if c == 0:
                                box[0] = kq_ps.tile(
                                    [P, NQ], F32, tag="kq", name=f"psb_{jj}"
                                )
                            psb = box[0]
                            nc.tensor.matmul(
                                psb[:, c * 512 : (c + 1) * 512],
                                lhsT=e2map_sb[:, :],
                                rhs=recip_all[:, jj, c * 512 : (c + 1) * 512],
                            )
                            if c == 1:
                                nc.vector.tensor_tensor(
                                    out=waT[:, jj, :],
                                    in0=waT[:, jj, :],
                                    in1=psb[:],
                                    op=ALU.mult,
                                )

                        PIECE_SLOTS = set(range(0, 2 * KT, 2)) | {1, 3}
                        for j in range(H // 2):
                            if j + 1 < H // 2:
                                wk_cur = load_w(j + 1, "k")
                                wq_cur = load_w(j + 1, "q")
                                pieces = [
                                    ("kq", qk, cp, pr)
                                    for qk, cp in (("k", 0), ("k", 1), ("q", 0))
                                    for pr in range(DT // 2)
                                ]
                            else:
                                pieces = [
                                    ("psb", jj, c)
                                    for jj in range(H // 2 - 1)
                                    for c in range(2)
                                ]
                            pc = 0
                            unit_box = [None]
                            den_j = dtmp_pool.tile(
                                [2, NQ], F32, tag="den", name=f"den_{j}"
                            )
                            for hh in range(2):
                                lo, hi = hh * DH, (hh + 1) * DH
                                hgl = (2 * j + hh) * (DH + 1)

                                def scores_mm(kt):
                                    pss = s_ps.tile(
                                        [P, NQ], F32, tag="s",
                                        name=f"s_{j}_{hh}_{kt}",
                                    )
                                    for c in range(QC):
                                        nc.tensor.matmul(
                                            pss[:, c * 512 : (c + 1) * 512],
                                            lhsT=k_sb[lo:hi, j, kt * P : (kt + 1) * P],
                                            rhs=q_sb[lo:hi, j, c * 512 : (c + 1) * 512],
                                            tile_position=(lo, 0),
                                        )
                                    pe_t = p_pool.tile([P, NQ], BF16, tag="pe")
                                    nc.scalar.activation(
                                        out=pe_t[:], in_=pss[:], func=AF.Exp, scale=SCALE
                                    )
                                    pt = p_pool.tile([P, NQ], BF16, tag="p")
                                    nc.vector.tensor_tensor(
                                        out=pt[:],
                                        in0=pe_t[:],
                                        in1=mask_sb[:, kt, :],
                                        op=ALU.mult,
                                    )
                                    return pt

                                av = av_ps.tile(
                                    [P, NQ], F32, tag="av", name=f"av_{j}_{hh}"
                                )
                                pt_cur = scores_mm(0)
                                for kt in range(KT):
                                    pt_next = scores_mm(kt + 1) if kt + 1 < KT else None
                                    if hh * KT + kt in PIECE_SLOTS and pc < len(pieces):
                                        piece = pieces[pc]
                                        if piece[0] == "kq":
                                            _, qk, cp, dt = piece
                                            kq_piece(
                                                j + 1,
                                                wk_cur if qk == "k" else wq_cur,
                                                qk, cp, dt, unit_box,
                                            )
                                        else:
                                            _, jj, c = piece
                                            psb_piece(jj, c, unit_box)
                                        pc += 1
                                    for c in range(QC):
                                        nc.tensor.matmul(
                                            av[0 : DH + 1, c * 512 : (c + 1) * 512],
                                            lhsT=v_sb[:, kt, hgl : hgl + DH + 1],
                                            rhs=pt_cur[:, c * 512 : (c + 1) * 512],
                                            start=(kt == 0),
                                            stop=(kt == KT - 1),
                                        )
                                    pt_cur = pt_next
                                nc.vector.tensor_copy(
                                    out=waT[hh * DH : (hh + 1) * DH, j, :],
                                    in_=av[0:DH, :],
                                )
                                dtmp = dtmp_pool.tile([P, NQ], F32, tag="dtmp")
                                nc.vector.tensor_copy(
                                    out=dtmp[DH : DH + 1, :], in_=av[DH : DH + 1, :]
                                )
                                (nc.sync if hh else nc.gpsimd).dma_start(
                                    out=den_j[hh : hh + 1, :], in_=dtmp[DH : DH + 1, :]
                                )
                            scr_j = scr_pool.tile([2, NQ], F32, tag="scr", name=f"scr_{j}")
                            nc.vector.reciprocal_approx_accurate(
                                out=recip_all[:, j, :], in_=den_j[:], scratch=scr_j[:]
                            )
                        box5 = [None]
                        psb_piece(H // 2 - 1, 0, box5)
                        psb_piece(H // 2 - 1, 1, box5)

        if stop_after is None:
            # ---------- phases D+E ----------
            with ExitStack() as s_de:
                de_pool = s_de.enter_context(tc.tile_pool(name="de", bufs=1))
                x2_sb = de_pool.tile([P, NQT, DIM], F32)
                h2T = de_pool.tile([P, DT, NQ], BF16)

                # ---------- phase D: normalize wa + proj + residual + LN2 ----------
                with ExitStack() as ph:
                    wp_pool = ph.enter_context(tc.tile_pool(name="wp", bufs=1))
                    xr_pool = ph.enter_context(tc.tile_pool(name="xr", bufs=3))
                    wproj_sb = wp_pool.tile([P, DT, DIM], BF16)
                    nc.gpsimd.dma_start(
                        out=wproj_sb[:],
                        in_=dwproj[:],
                    )
                    with ExitStack() as ph2:
                        pr_ps = ph2.enter_context(
                            tc.tile_pool(name="prps", bufs=3, space=bass.MemorySpace.PSUM)
                        )
                        for t in range(NQT):
                            xr = xr_pool.tile([P, DIM], F32, tag="xr")
                            nc.sync.dma_start(out=xr[:], in_=xv[:, t, :])
                            psp = pr_ps.tile([P, DIM], F32, tag="pr")
                            for dt in range(DT):
                                for c0, cw in ((0, 512), (512, 256)):
                                    nc.tensor.matmul(
                                        psp[:, c0 : c0 + cw],
                                        lhsT=waT[:, dt, t * P : (t + 1) * P],
                                        rhs=wproj_sb[:, dt, c0 : c0 + cw],
                                        start=(dt == 0),
                                        stop=(not with_bias and dt == DT - 1),
                                    )
                            if with_bias:
                                for c0, cw in ((0, 512), (512, 256)):
                                    nc.tensor.matmul(
                                        psp[:, c0 : c0 + cw],
                                        lhsT=r(ones_col[:, :]),
                                        rhs=r(bproj_row[:, c0 : c0 + cw]),
                                        start=False,
                                        stop=True,
                                    )
                            nc.vector.tensor_tensor(
                                out=x2_sb[:, t, :], in0=psp[:], in1=xr[:], op=ALU.add
                            )
                    h2_pool = ph.enter_context(tc.tile_pool(name="h2", bufs=4))
                    st2_pool = ph.enter_context(tc.tile_pool(name="st2", bufs=6))
                    tp2_pool = ph.enter_context(
                        tc.tile_pool(name="tp2", bufs=7, space=bass.MemorySpace.PSUM)
                    )
                    for tg in range(NQT // 4):
                        ps = [
                            tp2_pool.tile([P, 512], BF16, tag="tp2", name=f"tp2_{tg}_{i}")
                            for i in range(DT)
                        ]
                        for tt in range(4):
                            t = tg * 4 + tt
                            h2_t = h2_pool.tile([P, DIM], BF16, tag="h2")
                            layer_norm_tile(st2_pool, x2_sb[:, t, :], h2_t[:])
                            for dt in range(DT):
                                nc.tensor.transpose(
                                    ps[dt][:, tt * P : (tt + 1) * P],
                                    h2_t[:, dt * P : (dt + 1) * P],
                                    ident_bf[:],
                                )
                        for dt in range(DT):
                            nc.vector.tensor_copy(
                                out=h2T[:, dt, tg * 512 : (tg + 1) * 512], in_=ps[dt][:]
                            )

                # ---------- phase E: MLP ----------
                with ExitStack() as ph:
                    w1_pool = ph.enter_context(tc.tile_pool(name="w1p", bufs=4))
                    w2_pool = ph.enter_context(tc.tile_pool(name="w2p", bufs=4))
                    g_pool = ph.enter_context(tc.tile_pool(name="gp", bufs=1))
                    f_ps = ph.enter_context(
                        tc.tile_pool(name="fps", bufs=2 if not with_bias else 3, space=bass.MemorySpace.PSUM)
                    )
                    y_ps = ph.enter_context(
                        tc.tile_pool(name="yps", bufs=4, space=bass.MemorySpace.PSUM)
                    )
                    y_pool = ph.enter_context(tc.tile_pool(name="yp", bufs=5))
    
                    TQC = mlp_chunk // 512
                    NTC = mlp_chunk // P
                    for mc in range(MC):
                        q0 = mc * mlp_chunk
                        gT = g_pool.tile([P, HT, mlp_chunk], BF16, tag="g")
                        assert TQC == 1
                        # bias-free path merges gelu over hidden-tile pairs
                        # (per-partition bias differs across the pair, so the
                        # merged op is only valid with zero b1)
                        GHT = 1 if with_bias else 2
                        for ht in range(0, HT, GHT):
                            psf = f_ps.tile(
                                [P, 512 * GHT], F32, tag="f", name=f"psf_{mc}_{ht}"
                            )
                            for sub in range(GHT):
                                w1_t = w1_pool.tile([P, DT, P], BF16, tag="w1")
                                nc.gpsimd.dma_start(
                                    out=w1_t[:],
                                    in_=dw1[ht + sub].rearrange("p (dt o) -> p dt o", o=P),
                                )
                                for dt in range(DT):
                                    nc.tensor.matmul(
                                        psf[:, sub * 512 : (sub + 1) * 512],
                                        lhsT=w1_t[:, dt, :],
                                        rhs=h2T[:, dt, q0 : q0 + 512],
                                        start=(dt == 0),
                                        stop=(dt == DT - 1),
                                    )
                            nc.scalar.activation(
                                out=gT[:, ht : ht + GHT, :].rearrange(
                                    "p a b -> p (a b)"
                                ),
                                in_=psf[:],
                                func=AF.Gelu if gelu else AF.Identity,
                                bias=b1_pp[:, ht : ht + 1] if with_bias else 0.0,
                                scale=1.0,
                            )
                        y_ts = [
                            y_pool.tile([P, DIM], F32, tag="yt", name=f"yt_{mc}_{i}")
                            for i in range(NTC)
                        ]
                        for c0, cw in ((0, 512), (512, 256)):
                            psy = [
                                y_ps.tile([P, 512], F32, tag="y", name=f"psy_{mc}_{c0}_{i}")
                                for i in range(NTC)
                            ]
                            for ht in range(HT):
                                w2_t = w2_pool.tile([P, 512], BF16, tag="w2")
                                nc.sync.dma_start(
                                    out=w2_t[:, :cw],
                                    in_=dw2[ht * P : (ht + 1) * P, c0 : c0 + cw],
                                )
                                for t in range(NTC):
                                    nc.tensor.matmul(
                                        psy[t][:, :cw],
                                        lhsT=gT[:, ht, t * P : (t + 1) * P],
                                        rhs=w2_t[:, :cw],
                                        start=(ht == 0),
                                        stop=(not with_bias and ht == HT - 1),
                                    )
                            if with_bias:
                                for t in range(NTC):
                                    nc.tensor.matmul(
                                        psy[t][:, :cw],
                                        lhsT=r(ones_col[:, :]),
                                        rhs=r(b2_row[:, c0 : c0 + cw]),
                                        start=False,
                                        stop=True,
                                    )
                            for t in range(NTC):
                                tg = mc * NTC + t
                                nc.vector.tensor_tensor(
                                    out=y_ts[t][:, c0 : c0 + cw],
                                    in0=psy[t][:, :cw],
                                    in1=x2_sb[:, tg, c0 : c0 + cw],
                                    op=ALU.add,
                                )
                        for t in range(NTC):
                            nc.sync.dma_start(out=yv[:, mc * NTC + t, :], in_=y_ts[t][:])


        else:
            with ExitStack() as s_dummy:
                dpool = s_dummy.enter_context(tc.tile_pool(name="dumy", bufs=1))
                dt_ = dpool.tile([P, DIM], F32)
                nc.vector.memset(dt_[:], 0.0)
                for t in range(NQT):
                    nc.sync.dma_start(out=yv[:, t, :], in_=dt_[:])
    nc.compile()
    return nc


# ---------------- host-side preprocessing ----------------


def make_core_inputs(inp, core, S=2048, NQ=1024):
    b, half = core // 2, core % 2
    q0 = half * NQ
    x = np.asarray(inp["x"][b], np.float32)
    xrot = np.concatenate([x[q0 : q0 + NQ], x[:q0] if q0 else x[NQ:]], axis=0)
    mask = np.asarray(inp["mask"][b, 0], np.float32)
    mq = mask[q0 : q0 + NQ]
    mrot = np.concatenate(
        [mq[:, q0 : q0 + NQ], mq[:, :q0] if q0 else mq[:, NQ:]], axis=1
    )
    maskT = np.ascontiguousarray(mrot.T).astype(ml_dtypes.bfloat16)

    g1 = np.asarray(inp["g1"], np.float32)
    be1 = np.asarray(inp["beta1"], np.float32)
    g2 = np.asarray(inp["g2"], np.float32)
    be2 = np.asarray(inp["beta2"], np.float32)
    w_qkv = np.asarray(inp["w_qkv"], np.float32)
    wqkv = w_qkv * g1[:, None]
    bqkv = np.asarray(inp["b_qkv"], np.float32) + be1 @ w_qkv
    w1f = np.asarray(inp["w1"], np.float32)
    w1 = w1f * g2[:, None]
    b1 = np.asarray(inp["b1"], np.float32) + be2 @ w1f

    e2map = np.zeros((2, P), np.float32)
    e2map[0, :DH] = 1.0
    e2map[1, DH:] = 1.0

    KTl, NQTl = S // P, NQ // P
    xpk = np.ascontiguousarray(xrot.reshape(KTl, P, DIM).transpose(1, 0, 2))
    mpk = np.ascontiguousarray(maskT.reshape(KTl, P, NQ).transpose(1, 0, 2))
    F8NP = ml_dtypes.float8_e4m3
    wvr8 = np.ascontiguousarray(
        wqkv[:, 2 * DIM :].reshape(DT, P, DIM).transpose(1, 0, 2)
        .reshape(P, DT // 2, 2, DIM)
    ).astype(F8NP)
    wkqr8 = np.zeros((2 * DT, P, DT // 2, 2, P), F8NP)
    for m in range(DT):
        wkqr8[m] = (
            wqkv[:, DIM + m * P : DIM + (m + 1) * P]
            .reshape(DT, P, P).transpose(1, 0, 2).reshape(P, DT // 2, 2, P)
        ).astype(F8NP)
        wkqr8[DT + m] = (
            wqkv[:, m * P : (m + 1) * P]
            .reshape(DT, P, P).transpose(1, 0, 2).reshape(P, DT // 2, 2, P)
        ).astype(F8NP)
    wprojr = np.ascontiguousarray(
        np.asarray(inp["w_proj"], np.float32).reshape(DT, P, DIM).transpose(1, 0, 2)
    ).astype(ml_dtypes.bfloat16)
    HTl = HID // P
    w1r = np.ascontiguousarray(
        w1.reshape(DT, P, HTl, P).transpose(2, 1, 0, 3).reshape(HTl, P, DT * P)
    ).astype(ml_dtypes.bfloat16)
    return {
        "x": xpk,
        "maskT": mpk,
        "wvr8": wvr8,
        "wkqr8": wkqr8,
        "bqkv_pp": np.ascontiguousarray(bqkv.reshape(3 * DIM // P, P).T),
        "bv_row": bqkv[2 * DIM :].reshape(1, DIM).copy(),
        "wprojr": wprojr,
        "bproj_row": np.asarray(inp["b_proj"], np.float32).reshape(1, DIM).copy(),
        "w1r": w1r,
        "b1_pp": np.ascontiguousarray(b1.reshape(HID // P, P).T),
        "w2": np.asarray(inp["w2"], np.float32).astype(ml_dtypes.bfloat16),
        "b2_row": np.asarray(inp["b2"], np.float32).reshape(1, DIM).copy(),
        "e2map": e2map,
        "ones_row": np.ones((1, P), np.float32),
        "ident_bf": np.eye(P, dtype=ml_dtypes.bfloat16),
        "ident_f8": np.eye(P, dtype=ml_dtypes.float8_e4m3),
    }


def assemble_output(results, B=4, S=2048, NQ=1024):
    y = np.zeros((B, S, DIM), np.float32)
    for core, res in enumerate(results):
        b, half = core // 2, core % 2
        yr = res["y"].reshape(P, NQ // P, DIM).transpose(1, 0, 2).reshape(NQ, DIM)
        y[b, half * NQ : (half + 1) * NQ] = yr
    return y


# ---------------- harness entry point ----------------

_NC_CACHE = {}


def _get_nc(with_bias=True):
    key = ("nc", with_bias)
    if key not in _NC_CACHE:
        _NC_CACHE[key] = build_nc(gelu=True, with_bias=with_bias)
    return _NC_CACHE[key]


def needs_bias(in_maps):
    """True unless every in-kernel bias add is provably zero (the common
    case here: the extra bias matmuls + unmerged gelu are then skipped)."""
    m = in_maps[0]
    return any(
        np.any(np.asarray(m[k], np.float32))
        for k in ("bv_row", "bproj_row", "b1_pp", "b2_row")
    )


def kernel(**inputs):
    """Full (unsharded) inputs -> full (4, 2048, 768) float32 output.

    Shards batch x query-half across the 8 NeuronCores, runs the Bass/Tile
    kernel SPMD, and reassembles the output.
    """
    from concourse.bass_utils import run_bass_kernel_spmd

    in_maps = [make_core_inputs(inputs, c) for c in range(8)]
    nc = _get_nc(with_bias=needs_bias(in_maps))
    res = run_bass_kernel_spmd(nc, in_maps, core_ids=list(range(8)))
    return assemble_output(res.results)



# revision 43
# speedup vs baseline: 1.4726x; 1.0395x over previous
"""Bass/Tile kernel builder for the pre-LN attention block (dense_transformer).

Sharding: 8 cores = 4 batches x 2 query-halves. Each core:
  - loads x for its full batch; per 128-row block: LN1, transpose -> hT
    (dim-major, bf16), V matmuls for that block (PE fills DVE/DMA gaps)
  - attention per head-pair j with the K/Q matmuls for pair j+1 interleaved
    into the kt loop (PE fills the exp/mask bubbles; Act engine is the
    phase-C floor); scores kept transposed [k, q]: no max-subtraction
    (|score| <= ~9), denominator via ones-column appended to V
  - proj + residual (row-major), LN2, MLP, residual, store y rows

Dtypes: bf16 everywhere on the matmul paths (weights incl. proj/MLP), fp32
residuals/stats/denominators. Bias matmuls skipped when biases are all zero
(with_bias=False); host folds gamma/beta into weights either way.
SPMD trick: host rotates rows so each core's own rows are always [0, NQ).

PSUM budget: A: tp1(3x1)+qkvps(2x2)=7 banks; B/C: sps(2x2)+avps(2x2)=8,
K/Q units share the sps pool slots. D: nbps+prps; E: fps+yps.
"""

import sys

sys.path.insert(0, "/opt/trn_rl_repo")

from contextlib import ExitStack

import numpy as np
import ml_dtypes

import concourse.bass as bass
import concourse.tile as tile
import concourse.mybir as mybir
from concourse import bacc

F32 = mybir.dt.float32
F32R = mybir.dt.float32r
BF16 = mybir.dt.bfloat16
F8 = mybir.dt.float8e4
DR = mybir.MatmulPerfMode.DoubleRow
AF = mybir.ActivationFunctionType
ALU = mybir.AluOpType

DIM = 768
H = 12
DH = 64
HID = 3072
SCALE = DH ** -0.5
EPS = 1e-6
P = 128
DT = DIM // P


def r(x):
    return x.bitcast(F32R)


def build_nc(S=2048, NQ=1024, mlp_chunk=512, gelu=True, repeat=1, stop_after=None, with_bias=True):
    KT = S // P
    NQT = NQ // P
    assert NQ % 512 == 0
    QC = NQ // 512
    HT = HID // P
    MC = NQ // mlp_chunk
    KCW = min(1024, S)
    QCW = min(1024, NQ)

    nc = bacc.Bacc("TRN2", target_bir_lowering=False, debug=False, num_devices=8)

    dx = nc.dram_tensor("x", [P, S // P, DIM], F32, kind="ExternalInput").ap()
    dmask = nc.dram_tensor("maskT", [P, S // P, NQ], BF16, kind="ExternalInput").ap()
    dwv = nc.dram_tensor("wvr8", [P, DT // 2, 2, DIM], F8, kind="ExternalInput").ap()
    dwkq = nc.dram_tensor(
        "wkqr8", [2 * DT, P, DT // 2, 2, P], F8, kind="ExternalInput"
    ).ap()
    dbqkv = nc.dram_tensor("bqkv_pp", [P, 3 * DT], F32, kind="ExternalInput").ap()
    dbv = nc.dram_tensor("bv_row", [1, DIM], F32R, kind="ExternalInput").ap()
    dwproj = nc.dram_tensor("wprojr", [P, DT, DIM], BF16, kind="ExternalInput").ap()
    dbproj = nc.dram_tensor("bproj_row", [1, DIM], F32R, kind="ExternalInput").ap()
    dw1 = nc.dram_tensor("w1r", [HT, P, DT * P], BF16, kind="ExternalInput").ap()
    db1 = nc.dram_tensor("b1_pp", [P, HT], F32, kind="ExternalInput").ap()
    dw2 = nc.dram_tensor("w2", [HID, DIM], BF16, kind="ExternalInput").ap()
    db2 = nc.dram_tensor("b2_row", [1, DIM], F32R, kind="ExternalInput").ap()
    de2map = nc.dram_tensor("e2map", [2, P], F32, kind="ExternalInput").ap()
    dones = nc.dram_tensor("ones_row", [1, P], F32R, kind="ExternalInput").ap()
    dident_bf = nc.dram_tensor("ident_bf", [P, P], BF16, kind="ExternalInput").ap()
    dident_f8 = nc.dram_tensor("ident_f8", [P, P], F8, kind="ExternalInput").ap()
    dy = nc.dram_tensor("y", [P, NQ // P, DIM], F32, kind="ExternalOutput").ap()

    xv = dx
    maskv = dmask
    yv = dy

    with nc.allow_low_precision(
        reason="fp32r matmuls + bf16 attention path validated offline"
    ), tile.TileContext(nc) as tc, ExitStack() as top:
        rep_ctx = tc.For_i(0, repeat, 1) if repeat > 1 else ExitStack()
        top.enter_context(rep_ctx)
        consts = top.enter_context(tc.tile_pool(name="consts", bufs=1))
        ident_bf = consts.tile([P, P], BF16)
        nc.sync.dma_start(out=ident_bf[:], in_=dident_bf[:])
        ident_f8 = consts.tile([P, P], F8)
        nc.sync.dma_start(out=ident_f8[:], in_=dident_f8[:])
        e2map_sb = consts.tile([2, P], F32)
        nc.gpsimd.dma_start(out=e2map_sb[:], in_=de2map[:])
        eps_t = consts.tile([P, 1], F32)
        nc.vector.memset(eps_t[:], EPS)
        bqkv_pp = consts.tile([P, 3 * DT], F32)
        nc.gpsimd.dma_start(out=bqkv_pp[:], in_=dbqkv[:])
        if with_bias:
            ones_col = consts.tile([1, P], F32)
            nc.gpsimd.dma_start(out=r(ones_col[:]), in_=dones[:])
            bv_row = consts.tile([1, DIM], F32)
            nc.gpsimd.dma_start(out=r(bv_row[:]), in_=dbv[:])
            bproj_row = consts.tile([1, DIM], F32)
            nc.gpsimd.dma_start(out=r(bproj_row[:]), in_=dbproj[:])
            b1_pp = consts.tile([P, HT], F32)
            nc.gpsimd.dma_start(out=b1_pp[:], in_=db1[:])
            b2_row = consts.tile([1, DIM], F32)
            nc.gpsimd.dma_start(out=r(b2_row[:]), in_=db2[:])

        def layer_norm_tile(stats_pool, x_ap, out_ap):
            stats = stats_pool.tile([P, 2, 6], F32, tag="lnstats")
            for sg in range(2):
                nc.vector.bn_stats(
                    out=stats[:, sg, :], in_=x_ap[:, sg * 384 : (sg + 1) * 384]
                )
            mv = stats_pool.tile([P, 2], F32, tag="lnmv")
            nc.vector.bn_aggr(out=mv[:], in_=stats[:])
            sd = stats_pool.tile([P, 1], F32, tag="lnsd")
            nc.scalar.activation(
                out=sd[:], in_=mv[:, 1:2], func=AF.Sqrt, bias=eps_t[:], scale=1.0
            )
            rstd = stats_pool.tile([P, 1], F32, tag="lnrstd")
            nc.vector.reciprocal(out=rstd[:], in_=sd[:])
            nc.vector.tensor_scalar(
                out=out_ap,
                in0=x_ap,
                scalar1=mv[:, 0:1],
                scalar2=rstd[:],
                op0=ALU.subtract,
                op1=ALU.mult,
            )

        wa_pool = top.enter_context(tc.tile_pool(name="wa", bufs=1))
        waT = wa_pool.tile([P, DT, NQ], BF16)
        recip_all = wa_pool.tile([2, H // 2, NQ], F32)
        wproj_sb = wa_pool.tile([P, DT, DIM], BF16)
        nc.gpsimd.dma_start(out=wproj_sb[:], in_=dwproj[:])

        with ExitStack() as s_kqv:
            kqv_pool = s_kqv.enter_context(tc.tile_pool(name="kqv", bufs=1))
            k_sb = kqv_pool.tile([P, DT, S], BF16)
            q_sb = kqv_pool.tile([P, DT, NQ], BF16)
            v_sb = kqv_pool.tile([P, KT, H * (DH + 1)], BF16)
            mask_sb = kqv_pool.tile([P, KT, NQ], BF16)
            nc.gpsimd.dma_start(out=mask_sb[:], in_=maskv[:])
            v4 = v_sb.rearrange("p t (h s) -> p t h s", s=DH + 1)
            nc.vector.memset(v4[:, :, :, DH : DH + 1], 1.0)

            with ExitStack() as s_ht:
                ht_pool = s_ht.enter_context(tc.tile_pool(name="htp", bufs=1))
                hT = ht_pool.tile([P, DT, S], F8)

                # ---------- phase A: LN1 + transpose -> hT, V per block ----------
                with ExitStack() as ph:
                    wv_pool = ph.enter_context(tc.tile_pool(name="wv", bufs=1))
                    wv_sb = wv_pool.tile([P, DT // 2, 2, DIM], F8)
                    nc.gpsimd.dma_start(out=wv_sb[:], in_=dwv[:])
                    xo_pool = ph.enter_context(tc.tile_pool(name="xo", bufs=4))
                    h_pool = ph.enter_context(tc.tile_pool(name="h1", bufs=4))
                    st_pool = ph.enter_context(tc.tile_pool(name="st1", bufs=6))
                    tp_pool = ph.enter_context(
                        tc.tile_pool(name="tp1", bufs=3, space=bass.MemorySpace.PSUM)
                    )
                    qkv_ps = ph.enter_context(
                        tc.tile_pool(name="qkvps", bufs=2, space=bass.MemorySpace.PSUM)
                    )

                    def v_block(t):
                        psv = qkv_ps.tile([P, DIM], F32, tag="qkvps", name=f"psv_{t}")
                        for pr in range(DT // 2):
                            for c0, cw in ((0, 512), (512, 256)):
                                nc.tensor.matmul(
                                    psv[:, c0 : c0 + cw],
                                    lhsT=hT[:, 2 * pr : 2 * pr + 2, t * P : (t + 1) * P],
                                    rhs=wv_sb[:, pr, :, c0 : c0 + cw],
                                    start=(pr == 0),
                                    stop=(not with_bias and pr == DT // 2 - 1),
                                    perf_mode=DR,
                                )
                        if with_bias:
                            for c0, cw in ((0, 512), (512, 256)):
                                nc.tensor.matmul(
                                    psv[:, c0 : c0 + cw],
                                    lhsT=r(ones_col[:, :]),
                                    rhs=r(bv_row[:, c0 : c0 + cw]),
                                    start=False,
                                    stop=True,
                                )
                        nc.vector.tensor_copy(
                            out=v4[:, t, 0:H, 0:DH],
                            in_=psv[:].rearrange("p (h s) -> p h s", s=DH),
                        )

                    for t in range(KT):
                        xo = xo_pool.tile([P, DIM], F32, tag="xo")
                        nc.sync.dma_start(out=xo[:], in_=xv[:, t, :])
                        h_t = h_pool.tile([P, DIM], BF16, tag="h")
                        layer_norm_tile(st_pool, xo[:], h_t[:])
                        tp = tp_pool.tile([P, DIM], BF16, tag="tp", name=f"tp_{t}")
                        for dt in range(DT):
                            nc.tensor.transpose(
                                tp[:, dt * P : (dt + 1) * P],
                                h_t[:, dt * P : (dt + 1) * P],
                                ident_bf[:],
                            )
                        # bf16 -> fp8 cast rides the copy, on the idle Act
                        # engine (a DVE copy here would be 1x-rate and make
                        # phase A vector-bound)
                        nc.scalar.activation(
                            out=hT[:, :, t * P : (t + 1) * P],
                            in_=tp[:].rearrange("p (d o) -> p d o", o=P),
                            func=AF.Identity,
                            scale=1.0,
                        )
                        if t > 0:
                            v_block(t - 1)
                    v_block(KT - 1)

                if stop_after != "ab":
                    # ---------- phase B/C: attention, K/Q(j+1) interleaved ----------
                    # Per (j, hh) pass: scores computed one kt ahead of AV so
                    # the Act engine (exp, the phase floor) always has its
                    # next input ready; K/Q matmuls for pair j+1 are sliced
                    # into 2-matmul pieces dropped between scores and AV.
                    with ExitStack() as ph:
                        wqk_pool = ph.enter_context(tc.tile_pool(name="wqk", bufs=4))
                        s_ps = ph.enter_context(
                            tc.tile_pool(name="sps", bufs=2, space=bass.MemorySpace.PSUM)
                        )
                        av_ps = ph.enter_context(
                            tc.tile_pool(name="avps", bufs=1, space=bass.MemorySpace.PSUM)
                        )
                        kq_ps = ph.enter_context(
                            tc.tile_pool(name="kqps", bufs=1, space=bass.MemorySpace.PSUM)
                        )
                        p_pool = ph.enter_context(tc.tile_pool(name="pp", bufs=5))
                        dtmp_pool = ph.enter_context(tc.tile_pool(name="dtmp", bufs=2))
                        scr_pool = ph.enter_context(tc.tile_pool(name="scr", bufs=1))

                        def load_w(m, qk):
                            w = wqk_pool.tile(
                                [P, DT // 2, 2, P], F8, tag="wqk", name=f"w{qk}_{m}"
                            )
                            idx = m if qk == "k" else DT + m
                            nc.gpsimd.dma_start(out=w[:], in_=dwkq[idx])
                            return w

                        def kq_piece(m, w, qk, cp, pr, unit_box):
                            if pr == 0:
                                unit_box[0] = kq_ps.tile(
                                    [P, 1024], F32, tag="kq", name=f"{qk}u_{m}_{cp}"
                                )
                            ps = unit_box[0]
                            for ci in range(2):
                                c = cp * 2 + ci
                                nc.tensor.matmul(
                                    ps[:, ci * 512 : (ci + 1) * 512],
                                    lhsT=w[:, pr, :, :],
                                    rhs=hT[:, 2 * pr : 2 * pr + 2, c * 512 : (c + 1) * 512],
                                    start=(pr == 0),
                                    stop=(pr == DT // 2 - 1),
                                    perf_mode=DR,
                                )
                            if pr == DT // 2 - 1:
                                dst, boff = (k_sb, DT) if qk == "k" else (q_sb, 0)
                                nc.vector.tensor_scalar_add(
                                    out=dst[:, m, cp * 1024 : (cp + 1) * 1024],
                                    in0=ps[:],
                                    scalar1=bqkv_pp[:, boff + m : boff + m + 1],
                                )

                        def kq_full(m, w, qk, cp):
                            box = [None]
                            for pr in range(DT // 2):
                                kq_piece(m, w, qk, cp, pr, box)

                        wk_cur = load_w(0, "k")
                        wq_cur = load_w(0, "q")
                        kq_full(0, wk_cur, "k", 0)
                        kq_full(0, wk_cur, "k", 1)
                        kq_full(0, wq_cur, "q", 0)

                        def psb_piece(jj, c, box):
                            # normalize waT[:, jj] by its softmax denominators:
                            # e2map broadcasts the two recip rows to 128
                            # partitions (plain-fp32 matmul; fp32r would need
                            # an fp32r-rounded producer)
                            if c == 0:
                                box[0] = kq_ps.tile(
                                    [P, NQ], F32, tag="kq", name=f"psb_{jj}"
                                )
                            psb = box[0]
                            nc.tensor.matmul(
                                psb[:, c * 512 : (c + 1) * 512],
                                lhsT=e2map_sb[:, :],
                                rhs=recip_all[:, jj, c * 512 : (c + 1) * 512],
                            )
                            if c == 1:
                                nc.vector.tensor_tensor(
                                    out=waT[:, jj, :],
                                    in0=waT[:, jj, :],
                                    in1=psb[:],
                                    op=ALU.mult,
                                )

                        PIECE_SLOTS = set(range(0, 2 * KT, 2)) | {1, 3}
                        for j in range(H // 2):
                            if j + 1 < H // 2:
                                wk_cur = load_w(j + 1, "k")
                                wq_cur = load_w(j + 1, "q")
                                pieces = [
                                    ("kq", qk, cp, pr)
                                    for qk, cp in (("k", 0), ("k", 1), ("q", 0))
                                    for pr in range(DT // 2)
                                ]
                            else:
                                pieces = [
                                    ("psb", jj, c)
                                    for jj in range(H // 2 - 1)
                                    for c in range(2)
                                ]
                            pc = 0
                            unit_box = [None]
                            den_j = dtmp_pool.tile(
                                [2, NQ], F32, tag="den", name=f"den_{j}"
                            )
                            for hh in range(2):
                                lo, hi = hh * DH, (hh + 1) * DH
                                hgl = (2 * j + hh) * (DH + 1)

                                def scores_mm(kt):
                                    pss = s_ps.tile(
                                        [P, NQ], F32, tag="s",
                                        name=f"s_{j}_{hh}_{kt}",
                                    )
                                    for c in range(QC):
                                        nc.tensor.matmul(
                                            pss[:, c * 512 : (c + 1) * 512],
                                            lhsT=k_sb[lo:hi, j, kt * P : (kt + 1) * P],
                                            rhs=q_sb[lo:hi, j, c * 512 : (c + 1) * 512],
                                            tile_position=(lo, 0),
                                        )
                                    pe_t = p_pool.tile([P, NQ], BF16, tag="pe")
                                    nc.scalar.activation(
                                        out=pe_t[:], in_=pss[:], func=AF.Exp, scale=SCALE
                                    )
                                    pt = p_pool.tile([P, NQ], BF16, tag="p")
                                    nc.vector.tensor_tensor(
                                        out=pt[:],
                                        in0=pe_t[:],
                                        in1=mask_sb[:, kt, :],
                                        op=ALU.mult,
                                    )
                                    return pt

                                av = av_ps.tile(
                                    [P, NQ], F32, tag="av", name=f"av_{j}_{hh}"
                                )
                                pt_cur = scores_mm(0)
                                for kt in range(KT):
                                    pt_next = scores_mm(kt + 1) if kt + 1 < KT else None
                                    if hh * KT + kt in PIECE_SLOTS and pc < len(pieces):
                                        piece = pieces[pc]
                                        if piece[0] == "kq":
                                            _, qk, cp, dt = piece
                                            kq_piece(
                                                j + 1,
                                                wk_cur if qk == "k" else wq_cur,
                                                qk, cp, dt, unit_box,
                                            )
                                        else:
                                            _, jj, c = piece
                                            psb_piece(jj, c, unit_box)
                                        pc += 1
                                    for c in range(QC):
                                        nc.tensor.matmul(
                                            av[0 : DH + 1, c * 512 : (c + 1) * 512],
                                            lhsT=v_sb[:, kt, hgl : hgl + DH + 1],
                                            rhs=pt_cur[:, c * 512 : (c + 1) * 512],
                                            start=(kt == 0),
                                            stop=(kt == KT - 1),
                                        )
                                    pt_cur = pt_next
                                nc.vector.tensor_copy(
                                    out=waT[hh * DH : (hh + 1) * DH, j, :],
                                    in_=av[0:DH, :],
                                )
                                dtmp = dtmp_pool.tile([P, NQ], F32, tag="dtmp")
                                nc.vector.tensor_copy(
                                    out=dtmp[DH : DH + 1, :], in_=av[DH : DH + 1, :]
                                )
                                (nc.sync if hh else nc.gpsimd).dma_start(
                                    out=den_j[hh : hh + 1, :], in_=dtmp[DH : DH + 1, :]
                                )
                            scr_j = scr_pool.tile([2, NQ], F32, tag="scr", name=f"scr_{j}")
                            nc.vector.reciprocal_approx_accurate(
                                out=recip_all[:, j, :], in_=den_j[:], scratch=scr_j[:]
                            )
                        box5 = [None]
                        psb_piece(H // 2 - 1, 0, box5)
                        psb_piece(H // 2 - 1, 1, box5)

        if stop_after is None:
            # ---------- phases D+E ----------
            with ExitStack() as s_de:
                de_pool = s_de.enter_context(tc.tile_pool(name="de", bufs=1))
                x2_sb = de_pool.tile([P, NQT, DIM], F32)
                h2T = de_pool.tile([P, DT, NQ], BF16)

                # ---------- phase D: proj + residual + LN2 ----------
                with ExitStack() as ph:
                    xr_pool = ph.enter_context(tc.tile_pool(name="xr", bufs=3))
                    with ExitStack() as ph2:
                        pr_ps = ph2.enter_context(
                            tc.tile_pool(name="prps", bufs=3, space=bass.MemorySpace.PSUM)
                        )
                        for t in range(NQT):
                            xr = xr_pool.tile([P, DIM], F32, tag="xr")
                            nc.sync.dma_start(out=xr[:], in_=xv[:, t, :])
                            psp = pr_ps.tile([P, DIM], F32, tag="pr")
                            for dt in range(DT):
                                for c0, cw in ((0, 512), (512, 256)):
                                    nc.tensor.matmul(
                                        psp[:, c0 : c0 + cw],
                                        lhsT=waT[:, dt, t * P : (t + 1) * P],
                                        rhs=wproj_sb[:, dt, c0 : c0 + cw],
                                        start=(dt == 0),
                                        stop=(not with_bias and dt == DT - 1),
                                    )
                            if with_bias:
                                for c0, cw in ((0, 512), (512, 256)):
                                    nc.tensor.matmul(
                                        psp[:, c0 : c0 + cw],
                                        lhsT=r(ones_col[:, :]),
                                        rhs=r(bproj_row[:, c0 : c0 + cw]),
                                        start=False,
                                        stop=True,
                                    )
                            nc.vector.tensor_tensor(
                                out=x2_sb[:, t, :], in0=psp[:], in1=xr[:], op=ALU.add
                            )
                    h2_pool = ph.enter_context(tc.tile_pool(name="h2", bufs=4))
                    st2_pool = ph.enter_context(tc.tile_pool(name="st2", bufs=6))
                    tp2_pool = ph.enter_context(
                        tc.tile_pool(name="tp2", bufs=7, space=bass.MemorySpace.PSUM)
                    )
                    for tg in range(NQT // 4):
                        ps = [
                            tp2_pool.tile([P, 512], BF16, tag="tp2", name=f"tp2_{tg}_{i}")
                            for i in range(DT)
                        ]
                        for tt in range(4):
                            t = tg * 4 + tt
                            h2_t = h2_pool.tile([P, DIM], BF16, tag="h2")
                            layer_norm_tile(st2_pool, x2_sb[:, t, :], h2_t[:])
                            for dt in range(DT):
                                nc.tensor.transpose(
                                    ps[dt][:, tt * P : (tt + 1) * P],
                                    h2_t[:, dt * P : (dt + 1) * P],
                                    ident_bf[:],
                                )
                        for dt in range(DT):
                            nc.vector.tensor_copy(
                                out=h2T[:, dt, tg * 512 : (tg + 1) * 512], in_=ps[dt][:]
                            )

                # ---------- phase E: MLP ----------
                with ExitStack() as ph:
                    w1_pool = ph.enter_context(tc.tile_pool(name="w1p", bufs=4))
                    w2_pool = ph.enter_context(tc.tile_pool(name="w2p", bufs=4))
                    g_pool = ph.enter_context(tc.tile_pool(name="gp", bufs=1))
                    f_ps = ph.enter_context(
                        tc.tile_pool(name="fps", bufs=2 if not with_bias else 3, space=bass.MemorySpace.PSUM)
                    )
                    y_ps = ph.enter_context(
                        tc.tile_pool(name="yps", bufs=4, space=bass.MemorySpace.PSUM)
                    )
                    y_pool = ph.enter_context(tc.tile_pool(name="yp", bufs=5))
    
                    TQC = mlp_chunk // 512
                    NTC = mlp_chunk // P
                    for mc in range(MC):
                        q0 = mc * mlp_chunk
                        gT = g_pool.tile([P, HT, mlp_chunk], BF16, tag="g")
                        assert TQC == 1
                        # bias-free path merges gelu over hidden-tile pairs
                        # (per-partition bias differs across the pair, so the
                        # merged op is only valid with zero b1)
                        GHT = 1 if with_bias else 2
                        for ht in range(0, HT, GHT):
                            psf = f_ps.tile(
                                [P, 512 * GHT], F32, tag="f", name=f"psf_{mc}_{ht}"
                            )
                            for sub in range(GHT):
                                w1_t = w1_pool.tile([P, DT, P], BF16, tag="w1")
                                nc.gpsimd.dma_start(
                                    out=w1_t[:],
                                    in_=dw1[ht + sub].rearrange("p (dt o) -> p dt o", o=P),
                                )
                                for dt in range(DT):
                                    nc.tensor.matmul(
                                        psf[:, sub * 512 : (sub + 1) * 512],
                                        lhsT=w1_t[:, dt, :],
                                        rhs=h2T[:, dt, q0 : q0 + 512],
                                        start=(dt == 0),
                                        stop=(dt == DT - 1),
                                    )
                            nc.scalar.activation(
                                out=gT[:, ht : ht + GHT, :].rearrange(
                                    "p a b -> p (a b)"
                                ),
                                in_=psf[:],
                                func=AF.Gelu if gelu else AF.Identity,
                                bias=b1_pp[:, ht : ht + 1] if with_bias else 0.0,
                                scale=1.0,
                            )
                        y_ts = [
                            y_pool.tile([P, DIM], F32, tag="yt", name=f"yt_{mc}_{i}")
                            for i in range(NTC)
                        ]
                        for c0, cw in ((0, 512), (512, 256)):
                            psy = [
                                y_ps.tile([P, 512], F32, tag="y", name=f"psy_{mc}_{c0}_{i}")
                                for i in range(NTC)
                            ]
                            for ht in range(HT):
                                w2_t = w2_pool.tile([P, 512], BF16, tag="w2")
                                nc.sync.dma_start(
                                    out=w2_t[:, :cw],
                                    in_=dw2[ht * P : (ht + 1) * P, c0 : c0 + cw],
                                )
                                for t in range(NTC):
                                    nc.tensor.matmul(
                                        psy[t][:, :cw],
                                        lhsT=gT[:, ht, t * P : (t + 1) * P],
                                        rhs=w2_t[:, :cw],
                                        start=(ht == 0),
                                        stop=(not with_bias and ht == HT - 1),
                                    )
                            if with_bias:
                                for t in range(NTC):
                                    nc.tensor.matmul(
                                        psy[t][:, :cw],
                                        lhsT=r(ones_col[:, :]),
                                        rhs=r(b2_row[:, c0 : c0 + cw]),
                                        start=False,
                                        stop=True,
                                    )
                            for t in range(NTC):
                                tg = mc * NTC + t
                                nc.vector.tensor_tensor(
                                    out=y_ts[t][:, c0 : c0 + cw],
                                    in0=psy[t][:, :cw],
                                    in1=x2_sb[:, tg, c0 : c0 + cw],
                                    op=ALU.add,
                                )
                        for t in range(NTC):
                            nc.sync.dma_start(out=yv[:, mc * NTC + t, :], in_=y_ts[t][:])


        else:
            with ExitStack() as s_dummy:
                dpool = s_dummy.enter_context(tc.tile_pool(name="dumy", bufs=1))
                dt_ = dpool.tile([P, DIM], F32)
                nc.vector.memset(dt_[:], 0.0)
                for t in range(NQT):
                    nc.sync.dma_start(out=yv[:, t, :], in_=dt_[:])
    nc.compile()
    return nc


# ---------------- host-side preprocessing ----------------


def make_core_inputs(inp, core, S=2048, NQ=1024):
    b, half = core // 2, core % 2
    q0 = half * NQ
    x = np.asarray(inp["x"][b], np.float32)
    xrot = np.concatenate([x[q0 : q0 + NQ], x[:q0] if q0 else x[NQ:]], axis=0)
    mask = np.asarray(inp["mask"][b, 0], np.float32)
    mq = mask[q0 : q0 + NQ]
    mrot = np.concatenate(
        [mq[:, q0 : q0 + NQ], mq[:, :q0] if q0 else mq[:, NQ:]], axis=1
    )
    maskT = np.ascontiguousarray(mrot.T).astype(ml_dtypes.bfloat16)

    g1 = np.asarray(inp["g1"], np.float32)
    be1 = np.asarray(inp["beta1"], np.float32)
    g2 = np.asarray(inp["g2"], np.float32)
    be2 = np.asarray(inp["beta2"], np.float32)
    w_qkv = np.asarray(inp["w_qkv"], np.float32)
    wqkv = w_qkv * g1[:, None]
    bqkv = np.asarray(inp["b_qkv"], np.float32) + be1 @ w_qkv
    w1f = np.asarray(inp["w1"], np.float32)
    w1 = w1f * g2[:, None]
    b1 = np.asarray(inp["b1"], np.float32) + be2 @ w1f

    e2map = np.zeros((2, P), np.float32)
    e2map[0, :DH] = 1.0
    e2map[1, DH:] = 1.0

    KTl, NQTl = S // P, NQ // P
    xpk = np.ascontiguousarray(xrot.reshape(KTl, P, DIM).transpose(1, 0, 2))
    mpk = np.ascontiguousarray(maskT.reshape(KTl, P, NQ).transpose(1, 0, 2))
    F8NP = ml_dtypes.float8_e4m3
    wvr8 = np.ascontiguousarray(
        wqkv[:, 2 * DIM :].reshape(DT, P, DIM).transpose(1, 0, 2)
        .reshape(P, DT // 2, 2, DIM)
    ).astype(F8NP)
    wkqr8 = np.zeros((2 * DT, P, DT // 2, 2, P), F8NP)
    for m in range(DT):
        wkqr8[m] = (
            wqkv[:, DIM + m * P : DIM + (m + 1) * P]
            .reshape(DT, P, P).transpose(1, 0, 2).reshape(P, DT // 2, 2, P)
        ).astype(F8NP)
        wkqr8[DT + m] = (
            wqkv[:, m * P : (m + 1) * P]
            .reshape(DT, P, P).transpose(1, 0, 2).reshape(P, DT // 2, 2, P)
        ).astype(F8NP)
    wprojr = np.ascontiguousarray(
        np.asarray(inp["w_proj"], np.float32).reshape(DT, P, DIM).transpose(1, 0, 2)
    ).astype(ml_dtypes.bfloat16)
    HTl = HID // P
    w1r = np.ascontiguousarray(
        w1.reshape(DT, P, HTl, P).transpose(2, 1, 0, 3).reshape(HTl, P, DT * P)
    ).astype(ml_dtypes.bfloat16)
    return {
        "x": xpk,
        "maskT": mpk,
        "wvr8": wvr8,
        "wkqr8": wkqr8,
        "bqkv_pp": np.ascontiguousarray(bqkv.reshape(3 * DIM // P, P).T),
        "bv_row": bqkv[2 * DIM :].reshape(1, DIM).copy(),
        "wprojr": wprojr,
        "bproj_row": np.asarray(inp["b_proj"], np.float32).reshape(1, DIM).copy(),
        "w1r": w1r,
        "b1_pp": np.ascontiguousarray(b1.reshape(HID // P, P).T),
        "w2": np.asarray(inp["w2"], np.float32).astype(ml_dtypes.bfloat16),
        "b2_row": np.asarray(inp["b2"], np.float32).reshape(1, DIM).copy(),
        "e2map": e2map,
        "ones_row": np.ones((1, P), np.float32),
        "ident_bf": np.eye(P, dtype=ml_dtypes.bfloat16),
        "ident_f8": np.eye(P, dtype=ml_dtypes.float8_e4m3),
    }


def assemble_output(results, B=4, S=2048, NQ=1024):
    y = np.zeros((B, S, DIM), np.float32)
    for core, res in enumerate(results):
        b, half = core // 2, core % 2
        yr = res["y"].reshape(P, NQ // P, DIM).transpose(1, 0, 2).reshape(NQ, DIM)
        y[b, half * NQ : (half + 1) * NQ] = yr
    return y


# ---------------- harness entry point ----------------

_NC_CACHE = {}


def _get_nc(with_bias=True):
    key = ("nc", with_bias)
    if key not in _NC_CACHE:
        _NC_CACHE[key] = build_nc(gelu=True, with_bias=with_bias)
    return _NC_CACHE[key]


def needs_bias(in_maps):
    """True unless every in-kernel bias add is provably zero (the common
    case here: the extra bias matmuls + unmerged gelu are then skipped)."""
    m = in_maps[0]
    return any(
        np.any(np.asarray(m[k], np.float32))
        for k in ("bv_row", "bproj_row", "b1_pp", "b2_row")
    )


def kernel(**inputs):
    """Full (unsharded) inputs -> full (4, 2048, 768) float32 output.

    Shards batch x query-half across the 8 NeuronCores, runs the Bass/Tile
    kernel SPMD, and reassembles the output.
    """
    from concourse.bass_utils import run_bass_kernel_spmd

    in_maps = [make_core_inputs(inputs, c) for c in range(8)]
    nc = _get_nc(with_bias=needs_bias(in_maps))
    res = run_bass_kernel_spmd(nc, in_maps, core_ids=list(range(8)))
    return assemble_output(res.results)

